# revision 10
# baseline (speedup 1.0000x reference)
"""Trainium2 Bass kernel for DepthwiseTensorProductModuleDict.

Computes, for each key k in {a, b}:
    w = MLP(edge_len_k)           # Linear(64->128) -> LayerNorm -> silu -> Linear(128->256)
    out_k = DTP(edge_fea_k, edge_vec_k, w)   # depthwise uvu tensor product

Sharding: edge dimension split across 8 NeuronCores (pure data parallel),
both dict keys processed by every core on its edge shard. Weights replicated.

Layout: edges packed 4 per partition -> macro tiles of 512 edges
[128 partitions, 4 slots, features]. Per-macro pipeline:
  PE (bf16): 2x len transpose -> mm1 (N=128 + fused-mean N=2) ->
             4x a transpose -> mm2 (N=256 + N=128)
  ACT: len->bf16 cast, PSUM->SBUF copies, Square+accum (sum h^2),
       Sqrt (std), Silu(scale,bias) for layernorm+silu fusion
  DVE: fast-reciprocal (rstd), LN stats, PSUM-coupled DTP elementwise
  GPSIMD: SBUF-only DTP elementwise (contiguous APs only)
"""
import os
import numpy as np

import concourse.bass as bass
import concourse.tile as tile
from concourse import bacc, mybir
from concourse.bass_utils import run_bass_kernel_spmd
from concourse.masks import make_identity

F32 = mybir.dt.float32
BF16 = mybir.dt.bfloat16
P = 128          # partitions
J = 4            # edges per partition
MACRO = P * J    # 512 edges per macro tile
E = 131072       # total edges per key
NCORE = 8
ESH = E // NCORE          # 16384 edges per core per key
NM = ESH // MACRO         # 32 macros per key per core
MUL = 64
FEA = 256
RAD = 64
HID = 128
EPS = 1e-5

_mult = mybir.AluOpType.mult
_add = mybir.AluOpType.add
_sub = mybir.AluOpType.subtract

# cached compiled program (host-side) keyed by per-key flags
_CACHE = {}

last_exec_time_ns = None
last_results = None


def _prep_weights(W1, b1, W2):
    """Host-side weight packing (bf16 for the PE path).

    Returns:
      w1m  [64, 130] bf16: [W1 | mu_col | 0]
      w2a  [128, 256] bf16: [w3rep (192) | w4 (64)]   (pre-scaled)
      w2b  [128, 128] bf16: [w1 (64) | w2 (64)]       (pre-scaled)
      b1_nz flag
    """
    inv_s2 = np.float32(1.0 / np.sqrt(np.float32(2.0)))
    inv_s3 = np.float32(1.0 / np.sqrt(np.float32(3.0)))
    import ml_dtypes
    bf = ml_dtypes.bfloat16

    b1_nz = bool(np.any(b1))
    W1bf = W1.astype(bf).astype(np.float32)          # round first for consistency
    mu_col = W1bf.mean(axis=1, keepdims=True)        # [64, 1]
    pad = np.zeros_like(mu_col)
    w1m = np.hstack([W1bf, mu_col, pad]).astype(bf)        # [64, 130]

    w1 = W2[:, 0:64] * inv_s2
    w2 = W2[:, 64:128] * inv_s2
    w3 = W2[:, 128:192] * inv_s2
    w4 = W2[:, 192:256] * (inv_s2 * inv_s3)
    w3rep = np.repeat(w3, 3, axis=1)                       # [128, 192]
    w2a = np.concatenate([w3rep, w4], axis=1).astype(bf)   # [128, 256]
    w2b = np.concatenate([w1, w2], axis=1).astype(bf)      # [128, 128]
    return w1m, w2a, w2b, b1_nz


def _build_key(nc, tc, ctx, key, b1_nz, gbe_nz, ident_bf, eps_t, pools):
    """Emit instructions for one dict key's full shard (NM macros)."""
    fea = nc.dram_tensor(f"fea_{key}", [ESH, FEA], F32, kind="ExternalInput").ap()
    vec = nc.dram_tensor(f"vec_{key}", [ESH, 4], F32, kind="ExternalInput").ap()
    lng = nc.dram_tensor(f"len_{key}", [ESH, RAD], F32, kind="ExternalInput").ap()
    w1m_d = nc.dram_tensor(f"w1m_{key}", [RAD, 130], BF16,
                           kind="ExternalInput").ap()
    w2a_d = nc.dram_tensor(f"w2a_{key}", [HID, 256], BF16, kind="ExternalInput").ap()
    w2b_d = nc.dram_tensor(f"w2b_{key}", [HID, HID], BF16, kind="ExternalInput").ap()
    out = nc.dram_tensor(f"out_{key}", [ESH, FEA], F32, kind="ExternalOutput").ap()
    b1_d = g_d = be_d = None
    if b1_nz:
        b1_d = nc.dram_tensor(f"b1_{key}", [HID], F32, kind="ExternalInput").ap()
    if gbe_nz:
        g_d = nc.dram_tensor(f"g_{key}", [HID], F32, kind="ExternalInput").ap()
        be_d = nc.dram_tensor(f"be_{key}", [HID], F32, kind="ExternalInput").ap()

    fea_v = fea.rearrange("(m p j) f -> m p j f", p=P, j=J)
    len_v = lng.rearrange("(m p j) f -> m p (j f)", p=P, j=J)   # [NM, 128, 256]
    out_v = out.rearrange("(m p j) f -> m p j f", p=P, j=J)
    vec_v = vec.rearrange("(m p j) f -> p m (j f)", p=P, j=J)   # [128, NM, 16]

    const = ctx.enter_context(tc.tile_pool(name=f"const_{key}", bufs=1))

    # --- weights ---
    w1m_sb = const.tile([RAD, 130], BF16)
    nc.sync.dma_start(out=w1m_sb, in_=w1m_d)
    w2a_sb = const.tile([HID, 256], BF16)
    nc.sync.dma_start(out=w2a_sb, in_=w2a_d)
    w2b_sb = const.tile([HID, HID], BF16)
    nc.sync.dma_start(out=w2b_sb, in_=w2b_d)

    b1rep = b1mu = grep = berep = None
    if b1_nz:
        b1mu_d = nc.dram_tensor(f"b1mu_{key}", [P, 1], F32,
                                kind="ExternalInput").ap()
        b1rep = const.tile([P, HID], F32)
        nc.sync.dma_start(out=b1rep, in_=b1_d.partition_broadcast(P))
        b1mu = const.tile([P, 1], F32)
        nc.sync.dma_start(out=b1mu, in_=b1mu_d)
    if gbe_nz:
        grep = const.tile([P, HID], F32)
        berep = const.tile([P, HID], F32)
        nc.sync.dma_start(out=grep, in_=g_d.partition_broadcast(P))
        nc.sync.dma_start(out=berep, in_=be_d.partition_broadcast(P))

    # --- whole-shard vec resident in SBUF ---
    vec_sb = const.tile([P, NM, J * 4], F32)
    nc.sync.dma_start(out=vec_sb, in_=vec_v)

    io, wk, st, ps_misc, ps_h, ps_wa, ps_wb = pools

    for m in range(NM):
        # ---------- loads ----------
        len_t = io.tile([P, J * RAD], F32, name="len_t")
        nc.sync.dma_start(out=len_t, in_=len_v[m])
        fea_t = io.tile([P, J, FEA], F32, name="fea_t")
        nc.sync.dma_start(out=fea_t, in_=fea_v[m])

        vrow = vec_sb[:, m, :].rearrange("p (j f) -> p j f", f=4)   # [P,J,4]

        # ---------- len -> bf16, PE transpose (2 blocks), mm1 ----------
        len_bf = wk.tile([P, J * RAD], BF16, name="len_bf")
        nc.scalar.copy(len_bf, len_t)
        lt_ps = ps_misc.tile([RAD, J * P], BF16, name="lt_ps", tag="misc")
        for j in range(J):
            nc.tensor.transpose(lt_ps[:, j * P:(j + 1) * P],
                                len_bf[:, j * RAD:(j + 1) * RAD], ident_bf)
        lt_sb = wk.tile([RAD, J * P], BF16, name="lt_sb")
        nc.scalar.copy(lt_sb, lt_ps)

        # mm1: h (N=128) + fused mean (N=2), all stationaries at base 0
        h_ps = ps_h.tile([P, J, HID], F32, name="h_ps")
        mu_ps = ps_misc.tile([P, J, 2], F32, name="mu_ps", tag="misc")
        for j in range(J):
            slab = lt_sb[:, j * P:(j + 1) * P]
            nc.tensor.matmul(h_ps[:, j, :], slab, w1m_sb[:, 0:HID],
                             start=True, stop=True)
            nc.tensor.matmul(mu_ps[:, j, :], slab, w1m_sb[:, HID:HID + 2],
                             start=True, stop=True)
        if b1_nz:
            hb = wk.tile([P, J, HID], F32, name="hb")
            nc.vector.tensor_tensor(
                out=hb, in0=h_ps,
                in1=b1rep.unsqueeze(1).broadcast_to([P, J, HID]), op=_add)
            h_src = hb
        else:
            h_src = h_ps

        # ---------- layernorm stats ----------
        sq_d = wk.tile([P, J, HID], F32, name="sq_d")
        ssq = st.tile([P, J], F32, name="ssq")
        for j in range(J):
            nc.scalar.activation(sq_d[:, j], h_src[:, j, :],
                                 mybir.ActivationFunctionType.Square,
                                 accum_out=ssq[:, j:j + 1])
        mus = st.tile([P, J], F32, name="mus")
        nc.vector.tensor_copy(mus, mu_ps[:, :, 0:1].squeeze(2))
        if b1_nz:
            musb = st.tile([P, J], F32, name="musb")
            nc.vector.tensor_tensor(out=musb, in0=mus,
                                    in1=b1mu.broadcast_to([P, J]), op=_add)
            mus = musb
        musq = st.tile([P, J], F32, name="musq")
        nc.gpsimd.tensor_tensor(out=musq, in0=mus, in1=mus, op=_mult)
        var = st.tile([P, J], F32, name="var")
        nc.vector.scalar_tensor_tensor(out=var, in0=ssq, scalar=1.0 / HID,
                                       in1=musq, op0=_mult, op1=_sub)
        std = st.tile([P, J], F32, name="std")
        nc.scalar.activation(std, var, mybir.ActivationFunctionType.Sqrt,
                             bias=eps_t[:, 0:1])
        rstd = st.tile([P, J], F32, name="rstd")
        nc.vector.reciprocal(out=rstd, in_=std)
        nbias = st.tile([P, J], F32, name="nbias")
        nc.vector.scalar_tensor_tensor(out=nbias, in0=mus, scalar=-1.0,
                                       in1=rstd, op0=_mult, op1=_mult)

        # ---------- normalize + silu -> bf16 a ----------
        a_sb = wk.tile([P, J, HID], BF16, name="a_sb")
        if not gbe_nz:
            for j in range(J):
                nc.scalar.activation(a_sb[:, j], h_src[:, j, :],
                                     mybir.ActivationFunctionType.Silu,
                                     bias=nbias[:, j:j + 1],
                                     scale=rstd[:, j:j + 1])
        else:
            hn = wk.tile([P, J, HID], F32, name="hn")
            for j in range(J):
                nc.scalar.activation(hn[:, j], h_src[:, j, :],
                                     mybir.ActivationFunctionType.Identity,
                                     bias=nbias[:, j:j + 1],
                                     scale=rstd[:, j:j + 1])
            hg = wk.tile([P, J, HID], F32, name="hg")
            nc.vector.tensor_tensor(
                out=hg, in0=hn,
                in1=grep.unsqueeze(1).broadcast_to([P, J, HID]), op=_mult)
            nc.vector.tensor_tensor(
                out=hg, in0=hg,
                in1=berep.unsqueeze(1).broadcast_to([P, J, HID]), op=_add)
            for j in range(J):
                nc.scalar.activation(a_sb[:, j], hg[:, j],
                                     mybir.ActivationFunctionType.Silu)

        # ---------- PE: transpose a, mm2 ----------
        at_ps = ps_misc.tile([P, J, HID], BF16, name="at_ps", tag="misc")
        for j in range(J):
            nc.tensor.transpose(at_ps[:, j, :], a_sb[:, j, :], ident_bf)
        at_sb = wk.tile([P, J, HID], BF16, name="at_sb")
        nc.scalar.copy(at_sb, at_ps)

        wba = ps_wa.tile([P, J, 256], F32, name="wba")   # [w3rep|w4]
        wbb = ps_wb.tile([P, J, HID], F32, name="wbb")   # [w1|w2]
        for j in range(J):
            nc.tensor.matmul(wba[:, j, :], at_sb[:, j, :], w2a_sb,
                             start=True, stop=True)
            nc.tensor.matmul(wbb[:, j, :], at_sb[:, j, :], w2b_sb,
                             start=True, stop=True)

        # ---------- DTP ----------
        out_t = io.tile([P, J, FEA], F32, name="out_t")
        x0 = fea_t[:, :, 0:MUL]                    # [P,J,64]
        x1 = fea_t[:, :, MUL:FEA]                  # [P,J,192]
        x1v = x1.rearrange("p j (u d) -> p j u d", d=3)
        y1b = vrow[:, :, 1:4].unsqueeze(2).broadcast_to([P, J, MUL, 3])

        # b = x1 * y1b  (GPS, contiguous out)
        b_t = wk.tile([P, J, MUL, 3], F32, name="b_t")
        nc.gpsimd.tensor_tensor(out=b_t, in0=x1v, in1=y1b, op=_mult)

        # t2 = w2' * x0   (DVE, PSUM in0)
        t2 = wk.tile([P, J, MUL], F32, name="t2")
        nc.vector.tensor_tensor(out=t2, in0=wbb[:, :, 64:128], in1=x0, op=_mult)

        # e = t2b * y1b  (GPS, double-broadcast)
        e_t = wk.tile([P, J, MUL, 3], F32, name="e_t")
        nc.gpsimd.tensor_tensor(
            out=e_t, in0=t2.unsqueeze(3).broadcast_to([P, J, MUL, 3]),
            in1=y1b, op=_mult)

        # g = (x1 * y0) * w3rep'  (DVE STT per j; wba is PSUM)
        g_t = wk.tile([P, J, MUL * 3], F32, name="g_t")
        for j in range(J):
            nc.vector.scalar_tensor_tensor(
                out=g_t[:, j], in0=x1[:, j], scalar=vrow[:, j, 0:1],
                in1=wba[:, j, 0:192], op0=_mult, op1=_mult)

        # out1 = e + g  (DVE)
        nc.vector.tensor_tensor(
            out=out_t[:, :, MUL:FEA],
            in0=e_t.rearrange("p j u d -> p j (u d)"), in1=g_t, op=_add)

        # d = sum_d b   (strided adds: DVE + GPS)
        d_t = wk.tile([P, J, MUL], F32, name="d_t")
        nc.vector.tensor_tensor(out=d_t, in0=b_t[:, :, :, 0],
                                in1=b_t[:, :, :, 1], op=_add)
        d2_t = wk.tile([P, J, MUL], F32, name="d2_t")
        nc.gpsimd.tensor_tensor(out=d2_t, in0=d_t, in1=b_t[:, :, :, 2], op=_add)

        # m1y = (x0*y0)*w1'  (DVE STT per j)
        m1y = wk.tile([P, J, MUL], F32, name="m1y")
        for j in range(J):
            nc.vector.scalar_tensor_tensor(
                out=m1y[:, j], in0=x0[:, j], scalar=vrow[:, j, 0:1],
                in1=wbb[:, j, 0:64], op0=_mult, op1=_mult)
        # md = d * w4'  (DVE, PSUM in1)
        md = wk.tile([P, J, MUL], F32, name="md")
        nc.vector.tensor_tensor(out=md, in0=d2_t, in1=wba[:, :, 192:256],
                                op=_mult)
        # out0 = m1y + md  (GPS)
        nc.gpsimd.tensor_tensor(out=out_t[:, :, 0:MUL], in0=m1y, in1=md, op=_add)

        # ---------- store ----------
        nc.sync.dma_start(out=out_v[m], in_=out_t)


def _build_program(flags):
    """flags = {key: (b1_nz, gbe_nz)}"""
    import contextlib
    nc = bacc.Bacc("TRN2", target_bir_lowering=False, debug=False)
    with tile.TileContext(nc) as tc:
        with contextlib.ExitStack() as ctx:
            glob = ctx.enter_context(tc.tile_pool(name="glob", bufs=1))
            ident = glob.tile([P, P], F32)
            make_identity(nc, ident)
            ident_bf = glob.tile([P, P], BF16)
            nc.scalar.copy(ident_bf, ident)
            eps_t = glob.tile([P, 1], F32)
            nc.vector.memset(eps_t, EPS)
            pools = (
                ctx.enter_context(tc.tile_pool(name="io", bufs=3)),
                ctx.enter_context(tc.tile_pool(name="wk", bufs=2)),
                ctx.enter_context(tc.tile_pool(name="st", bufs=2)),
                ctx.enter_context(tc.tile_pool(name="psmisc", bufs=1,
                                               space="PSUM")),
                ctx.enter_context(tc.tile_pool(name="psh", bufs=1, space="PSUM")),
                ctx.enter_context(tc.tile_pool(name="pswa", bufs=2, space="PSUM")),
                ctx.enter_context(tc.tile_pool(name="pswb", bufs=2, space="PSUM")),
            )
            for key in ("a", "b"):
                b1_nz, gbe_nz = flags[key]
                _build_key(nc, tc, ctx, key, b1_nz, gbe_nz, ident_bf, eps_t, pools)
    nc.compile()
    return nc


def kernel(edge_fea_a, edge_vec_a, edge_len_a, W1_a, b1_a, g_a, be_a, W2_a,
           edge_fea_b, edge_vec_b, edge_len_b, W1_b, b1_b, g_b, be_b, W2_b):
    global last_exec_time_ns, last_results
    ins = {
        "a": (edge_fea_a, edge_vec_a, edge_len_a, W1_a, b1_a, g_a, be_a, W2_a),
        "b": (edge_fea_b, edge_vec_b, edge_len_b, W1_b, b1_b, g_b, be_b, W2_b),
    }
    prepped = {}
    flags = {}
    for key, (fea, vec, lng, W1, b1, g, be, W2) in ins.items():
        w1m, w2a, w2b, b1_nz = _prep_weights(
            np.asarray(W1, np.float32), np.asarray(b1, np.float32),
            np.asarray(W2, np.float32))
        gbe_nz = bool(np.any(np.asarray(g) != 1.0) or np.any(np.asarray(be)))
        prepped[key] = (w1m, w2a, w2b)
        flags[key] = (b1_nz, gbe_nz)

    ck = tuple(flags[k] for k in ("a", "b"))
    if ck not in _CACHE:
        _CACHE[ck] = _build_program(flags)
    nc = _CACHE[ck]

    in_maps = []
    for c in range(NCORE):
        sl = slice(c * ESH, (c + 1) * ESH)
        m = {}
        for key, (fea, vec, lng, W1, b1, g, be, W2) in ins.items():
            m[f"fea_{key}"] = np.ascontiguousarray(np.asarray(fea, np.float32)[sl])
            m[f"vec_{key}"] = np.ascontiguousarray(np.asarray(vec, np.float32)[sl])
            m[f"len_{key}"] = np.ascontiguousarray(np.asarray(lng, np.float32)[sl])
            m[f"w1m_{key}"] = prepped[key][0]
            m[f"w2a_{key}"] = prepped[key][1]
            m[f"w2b_{key}"] = prepped[key][2]
            if flags[key][0]:
                m[f"b1_{key}"] = np.asarray(b1, np.float32)
                m[f"b1mu_{key}"] = np.full(
                    (P, 1), np.asarray(b1, np.float32).mean(), np.float32)
            if flags[key][1]:
                m[f"g_{key}"] = np.asarray(g, np.float32)
                m[f"be_{key}"] = np.asarray(be, np.float32)
        in_maps.append(m)

    trace = bool(int(os.environ.get("KERNEL_TRACE", "0")))
    res = run_bass_kernel_spmd(nc, in_maps, list(range(NCORE)), trace=trace)
    globals()["last_results"] = res
    last_exec_time_ns = res.exec_time_ns

    out_a = np.concatenate([np.asarray(res.results[c]["out_a"])
                            for c in range(NCORE)], axis=0)
    out_b = np.concatenate([np.asarray(res.results[c]["out_b"])
                            for c in range(NCORE)], axis=0)
    return (out_a, out_b)


# revision 13
# speedup vs baseline: 1.2200x; 1.2200x over previous
"""Trainium2 Bass kernel for DepthwiseTensorProductModuleDict.

Computes, for each key k in {a, b}:
    w = MLP(edge_len_k)           # Linear(64->128) -> LayerNorm -> silu -> Linear(128->256)
    out_k = DTP(edge_fea_k, edge_vec_k, w)   # depthwise uvu tensor product

Sharding: edge dimension split across 8 NeuronCores (pure data parallel),
both dict keys processed by every core on its edge shard. Weights replicated.

Layout: edges packed 4 per partition -> macro tiles of 512 edges
[128 partitions, 4 slots, features]. Per-macro pipeline:
  PE (bf16): 2x len transpose -> mm1 (N=128 + fused-mean N=2) ->
             4x a transpose -> mm2 (N=256 + N=128)
  ACT: len->bf16 cast, PSUM->SBUF copies, Square+accum (sum h^2),
       Sqrt (std), Silu(scale,bias) for layernorm+silu fusion
  DVE: fast-reciprocal (rstd), LN stats, PSUM-coupled DTP elementwise
  GPSIMD: SBUF-only DTP elementwise (contiguous APs only)
"""
import os
import numpy as np

import concourse.bass as bass
import concourse.tile as tile
from concourse import bacc, mybir
from concourse.bass_utils import run_bass_kernel_spmd
from concourse.masks import make_identity

F32 = mybir.dt.float32
BF16 = mybir.dt.bfloat16
I32 = mybir.dt.int32
P = 128          # partitions
J = 4            # edges per partition
MACRO = P * J    # 512 edges per macro tile
E = 131072       # total edges per key
NCORE = 8
ESH = E // NCORE          # 16384 edges per core per key
NM = ESH // MACRO         # 32 macros per key per core
MUL = 64
FEA = 256
RAD = 64
HID = 128
EPS = 1e-5

_mult = mybir.AluOpType.mult
_add = mybir.AluOpType.add
_sub = mybir.AluOpType.subtract

# cached compiled program (host-side) keyed by per-key flags
_CACHE = {}

last_exec_time_ns = None
last_results = None


def _prep_weights(W1, b1, W2):
    """Host-side weight packing (bf16 for the PE path).

    Returns:
      w1m   [64, 128] bf16: W1
      m2    [64, 132] bf16: [mu_col | 0 | W1 W1^T] split hi|lo (PSUM-accumulated
            for ~16-bit mantissa: mean + sum-h^2 quadratic form)
      w2big [128, 384] bf16: [w1 | w2 | w3rep | w4]  (pre-scaled)
      b1_nz flag
    """
    inv_s2 = np.float32(1.0 / np.sqrt(np.float32(2.0)))
    inv_s3 = np.float32(1.0 / np.sqrt(np.float32(3.0)))
    import ml_dtypes
    bf = ml_dtypes.bfloat16

    b1_nz = bool(np.any(b1))
    W1bf = W1.astype(bf).astype(np.float32)          # round first for consistency
    w1m = W1bf.astype(bf)                            # [64, 128]
    mu_col = W1bf.mean(axis=1, keepdims=True)        # [64, 1]
    pad = np.zeros_like(mu_col)
    M = W1bf @ W1bf.T                                # [64, 64] quadratic form
    m2f = np.hstack([mu_col, pad, M]).astype(np.float32)   # [64, 66]
    m2hi = m2f.astype(bf)
    m2lo = (m2f - m2hi.astype(np.float32)).astype(bf)
    m2 = np.hstack([m2hi, m2lo])                           # [64, 132] bf16

    w1 = W2[:, 0:64] * inv_s2
    w2 = W2[:, 64:128] * inv_s2
    w3 = W2[:, 128:192] * inv_s2
    w4 = W2[:, 192:256] * (inv_s2 * inv_s3)
    w3rep = np.repeat(w3, 3, axis=1)                       # [128, 192]
    w2big = np.concatenate([w1, w2, w3rep, w4], axis=1).astype(bf)  # [128, 384]
    return w1m, m2, w2big, b1_nz


def _build_key(nc, tc, ctx, key, b1_nz, gbe_nz, ident_bf, magic8, pools):
    """Emit instructions for one dict key's full shard (NM macros)."""
    fea = nc.dram_tensor(f"fea_{key}", [ESH, FEA], F32, kind="ExternalInput").ap()
    vec = nc.dram_tensor(f"vec_{key}", [ESH, 4], F32, kind="ExternalInput").ap()
    lng = nc.dram_tensor(f"len_{key}", [ESH, RAD], F32, kind="ExternalInput").ap()
    w1m_d = nc.dram_tensor(f"w1m_{key}", [RAD, HID], BF16,
                           kind="ExternalInput").ap()
    m2_d = nc.dram_tensor(f"m2_{key}", [RAD, 132], BF16,
                          kind="ExternalInput").ap()
    w2big_d = nc.dram_tensor(f"w2big_{key}", [HID, 384], BF16,
                             kind="ExternalInput").ap()
    out = nc.dram_tensor(f"out_{key}", [ESH, FEA], F32, kind="ExternalOutput").ap()
    b1_d = g_d = be_d = None
    if b1_nz:
        b1_d = nc.dram_tensor(f"b1_{key}", [HID], F32, kind="ExternalInput").ap()
    if gbe_nz:
        g_d = nc.dram_tensor(f"g_{key}", [HID], F32, kind="ExternalInput").ap()
        be_d = nc.dram_tensor(f"be_{key}", [HID], F32, kind="ExternalInput").ap()

    fea_v = fea.rearrange("(m p j) f -> m p j f", p=P, j=J)
    len_v = lng.rearrange("(m p j) f -> m p (j f)", p=P, j=J)   # [NM, 128, 256]
    out_v = out.rearrange("(m p j) f -> m p j f", p=P, j=J)
    vec_v = vec.rearrange("(m p j) f -> p m (j f)", p=P, j=J)   # [128, NM, 16]

    const = ctx.enter_context(tc.tile_pool(name=f"const_{key}", bufs=1))

    # --- weights ---
    w1m_sb = const.tile([RAD, HID], BF16)
    nc.sync.dma_start(out=w1m_sb, in_=w1m_d)
    m2_sb = const.tile([RAD, 132], BF16)
    nc.sync.dma_start(out=m2_sb, in_=m2_d)
    w2big_sb = const.tile([HID, 384], BF16)
    nc.sync.dma_start(out=w2big_sb, in_=w2big_d)

    b1rep = b1mu = grep = berep = None
    if b1_nz:
        b1mu_d = nc.dram_tensor(f"b1mu_{key}", [P, 1], F32,
                                kind="ExternalInput").ap()
        b1rep = const.tile([P, HID], F32)
        nc.sync.dma_start(out=b1rep, in_=b1_d.partition_broadcast(P))
        b1mu = const.tile([P, 1], F32)
        nc.sync.dma_start(out=b1mu, in_=b1mu_d)
    if gbe_nz:
        grep = const.tile([P, HID], F32)
        berep = const.tile([P, HID], F32)
        nc.sync.dma_start(out=grep, in_=g_d.partition_broadcast(P))
        nc.sync.dma_start(out=berep, in_=be_d.partition_broadcast(P))

    # --- whole-shard vec resident in SBUF ---
    vec_sb = const.tile([P, NM, J * 4], F32)
    nc.sync.dma_start(out=vec_sb, in_=vec_v)

    io, wk, st, ps_misc, ps_h, ps_wb = pools

    PAIR = 2
    for mp in range(NM // PAIR):
        ssq_p = st.tile([P, PAIR, J], F32, name="ssq_p")
        mus_p = st.tile([P, PAIR, J], F32, name="mus_p")
        saved = []
        for pm in range(PAIR):
            m = mp * PAIR + pm
            # ---------- loads ----------
            len_t = io.tile([P, J * RAD], F32, name="len_t")
            nc.sync.dma_start(out=len_t, in_=len_v[m])
            fea_t = io.tile([P, J, FEA], F32, name="fea_t")
            nc.sync.dma_start(out=fea_t, in_=fea_v[m])
            vrow = vec_sb[:, m, :].rearrange("p (j f) -> p j f", f=4)

            # ---------- len -> bf16, PE transpose, mm1 + [mu|M] ----------
            len_bf = wk.tile([P, J * RAD], BF16, name="len_bf")
            nc.scalar.copy(len_bf, len_t)
            lt_ps = ps_misc.tile([RAD, J * P], BF16, name="lt_ps", tag="misc")
            for j in range(J):
                nc.tensor.transpose(lt_ps[:, j * P:(j + 1) * P],
                                    len_bf[:, j * RAD:(j + 1) * RAD], ident_bf)
            lt_sb = wk.tile([RAD, J * P], BF16, name="lt_sb")
            nc.scalar.copy(lt_sb, lt_ps)

            h_ps = ps_h.tile([P, J, HID], F32, name="h_ps")
            mu_ps = ps_misc.tile([P, J, 66], F32, name="mu_ps", tag="misc")
            for j in range(J):
                slab = lt_sb[:, j * P:(j + 1) * P]
                nc.tensor.matmul(h_ps[:, j, :], slab, w1m_sb,
                                 start=True, stop=True)
                nc.tensor.matmul(mu_ps[:, j, :], slab, m2_sb[:, 0:66],
                                 start=True, stop=False)
                nc.tensor.matmul(mu_ps[:, j, :], slab, m2_sb[:, 66:132],
                                 start=False, stop=True)
            if b1_nz:
                hb = wk.tile([P, J, HID], F32, name="hb")
                nc.vector.tensor_tensor(
                    out=hb, in0=h_ps,
                    in1=b1rep.unsqueeze(1).broadcast_to([P, J, HID]), op=_add)
                h_src = hb
            else:
                h_src = h_ps

            # ssq = sum_i h_i^2 = sum_l (len @ M) * len   (quadratic form)
            qprod = wk.tile([P, J, RAD], F32, name="qprod")
            nc.vector.tensor_tensor(
                out=qprod, in0=mu_ps[:, :, 2:66],
                in1=len_bf.rearrange("p (j r) -> p j r", r=RAD), op=_mult)
            nc.vector.tensor_reduce(ssq_p[:, pm, :], qprod,
                                    axis=mybir.AxisListType.X,
                                    op=_add)
            nc.vector.tensor_copy(mus_p[:, pm, :], mu_ps[:, :, 0:1].squeeze(2))
            saved.append((m, fea_t, h_src, vrow))

        # ---------- layernorm stats (batched across the pair) ----------
        if b1_nz:
            musb = st.tile([P, PAIR, J], F32, name="musb")
            nc.vector.tensor_tensor(out=musb, in0=mus_p,
                                    in1=b1mu.broadcast_to([P, PAIR, J]),
                                    op=_add)
            mus_p = musb
        musq = st.tile([P, PAIR, J], F32, name="musq")
        nc.gpsimd.tensor_tensor(out=musq, in0=mus_p, in1=mus_p, op=_mult)
        var = st.tile([P, PAIR, J], F32, name="var")
        nc.vector.scalar_tensor_tensor(out=var, in0=ssq_p, scalar=1.0 / HID,
                                       in1=musq, op0=_mult, op1=_sub)
        # rstd via 1 Newton iteration from the fast-inverse-sqrt seed
        vpe = st.tile([P, PAIR, J], F32, name="vpe")
        nc.vector.tensor_scalar(out=vpe, in0=var, scalar1=EPS, scalar2=None,
                                op0=_add)
        nvpe = st.tile([P, PAIR, J], F32, name="nvpe")
        nc.vector.tensor_scalar(out=nvpe, in0=var, scalar1=-0.5,
                                scalar2=-EPS / 2, op0=_mult, op1=_add)
        ibits = st.tile([P, PAIR, J], I32, name="ibits")
        nc.vector.tensor_scalar(out=ibits, in0=vpe.bitcast(I32), scalar1=1,
                                scalar2=None,
                                op0=mybir.AluOpType.logical_shift_right)
        seed = st.tile([P, PAIR, J], I32, name="seed")
        nc.vector.tensor_tensor(out=seed,
                                in0=magic8.rearrange("p (a j) -> p a j", a=2),
                                in1=ibits, op=_sub)
        y2 = st.tile([P, PAIR, J], F32, name="y2")
        nc.gpsimd.tensor_tensor(out=y2, in0=seed.bitcast(F32),
                                in1=seed.bitcast(F32), op=_mult)
        w_ = st.tile([P, PAIR, J], F32, name="w_")
        nc.gpsimd.tensor_tensor(out=w_, in0=y2, in1=nvpe, op=_mult)
        y_a = st.tile([P, PAIR, J], F32, name="y_a")
        nc.vector.scalar_tensor_tensor(out=y_a, in0=w_, scalar=1.5,
                                       in1=seed.bitcast(F32), op0=_add,
                                       op1=_mult)
        # second Newton iteration (cheap on GPS, keeps rstd ~1e-6)
        y2b = st.tile([P, PAIR, J], F32, name="y2b")
        nc.gpsimd.tensor_tensor(out=y2b, in0=y_a, in1=y_a, op=_mult)
        w2b_ = st.tile([P, PAIR, J], F32, name="w2b_")
        nc.gpsimd.tensor_tensor(out=w2b_, in0=y2b, in1=nvpe, op=_mult)
        rstd = st.tile([P, PAIR, J], F32, name="rstd")
        nc.vector.scalar_tensor_tensor(out=rstd, in0=w2b_, scalar=1.5,
                                       in1=y_a, op0=_add, op1=_mult)
        nbias = st.tile([P, PAIR, J], F32, name="nbias")
        nc.vector.scalar_tensor_tensor(out=nbias, in0=mus_p, scalar=-1.0,
                                       in1=rstd, op0=_mult, op1=_mult)

        for pm in range(PAIR):
            m, fea_t, h_src, vrow = saved[pm]
            # ---------- normalize + silu -> bf16 a ----------
            a_sb = wk.tile([P, J, HID], BF16, name="a_sb")
            if not gbe_nz:
                for j in range(J):
                    nc.scalar.activation(a_sb[:, j], h_src[:, j, :],
                                         mybir.ActivationFunctionType.Silu,
                                         bias=nbias[:, pm, j:j + 1],
                                         scale=rstd[:, pm, j:j + 1])
            else:
                hn = wk.tile([P, J, HID], F32, name="hn")
                for j in range(J):
                    nc.scalar.activation(hn[:, j], h_src[:, j, :],
                                         mybir.ActivationFunctionType.Identity,
                                         bias=nbias[:, pm, j:j + 1],
                                         scale=rstd[:, pm, j:j + 1])
                hg = wk.tile([P, J, HID], F32, name="hg")
                nc.vector.tensor_tensor(
                    out=hg, in0=hn,
                    in1=grep.unsqueeze(1).broadcast_to([P, J, HID]), op=_mult)
                nc.vector.tensor_tensor(
                    out=hg, in0=hg,
                    in1=berep.unsqueeze(1).broadcast_to([P, J, HID]), op=_add)
                for j in range(J):
                    nc.scalar.activation(a_sb[:, j], hg[:, j],
                                         mybir.ActivationFunctionType.Silu)

            # ---------- PE: transpose a, mm2 (single N=384) ----------
            at_ps = ps_misc.tile([P, J, HID], BF16, name="at_ps", tag="misc")
            for j in range(J):
                nc.tensor.transpose(at_ps[:, j, :], a_sb[:, j, :], ident_bf)
            at_sb = wk.tile([P, J, HID], BF16, name="at_sb")
            nc.scalar.copy(at_sb, at_ps)

            wb = ps_wb.tile([P, J, 384], F32, name="wb",
                            padded_shape=[P, J, 512])
            for j in range(J):
                nc.tensor.matmul(wb[:, j, :], at_sb[:, j, :], w2big_sb,
                                 start=True, stop=True)

            # ---------- DTP ----------
            out_t = io.tile([P, J, FEA], F32, name="out_t")
            x0 = fea_t[:, :, 0:MUL]                    # [P,J,64]
            x1 = fea_t[:, :, MUL:FEA]                  # [P,J,192]
            x1v = x1.rearrange("p j (u d) -> p j u d", d=3)
            y1b = vrow[:, :, 1:4].unsqueeze(2).broadcast_to([P, J, MUL, 3])

            # b = x1 * y1b  (GPS)
            b_t = wk.tile([P, J, MUL, 3], F32, name="b_t")
            nc.gpsimd.tensor_tensor(out=b_t, in0=x1v, in1=y1b, op=_mult)
            # d = sum_d b   (strided adds: DVE + GPS)
            d_t = wk.tile([P, J, MUL], F32, name="d_t")
            nc.vector.tensor_tensor(out=d_t, in0=b_t[:, :, :, 0],
                                    in1=b_t[:, :, :, 1], op=_add)
            d2_t = wk.tile([P, J, MUL], F32, name="d2_t")
            nc.gpsimd.tensor_tensor(out=d2_t, in0=d_t, in1=b_t[:, :, :, 2],
                                    op=_add)

            # t2 = w2' * x0   (DVE, PSUM in0)
            t2 = wk.tile([P, J, MUL], F32, name="t2")
            nc.vector.tensor_tensor(out=t2, in0=wb[:, :, 64:128], in1=x0,
                                    op=_mult)
            # m1y = (x0*y0)*w1'  (DVE STT per j)
            m1y = wk.tile([P, J, MUL], F32, name="m1y")
            for j in range(J):
                nc.vector.scalar_tensor_tensor(
                    out=m1y[:, j], in0=x0[:, j], scalar=vrow[:, j, 0:1],
                    in1=wb[:, j, 0:64], op0=_mult, op1=_mult)
            # md = d * w4'  (DVE, PSUM in1)
            md = wk.tile([P, J, MUL], F32, name="md")
            nc.vector.tensor_tensor(out=md, in0=d2_t, in1=wb[:, :, 320:384],
                                    op=_mult)
            # g = (x1 * y0) * w3rep'  (DVE STT per j; wb is PSUM)
            g_t = wk.tile([P, J, MUL * 3], F32, name="g_t")
            for j in range(J):
                nc.vector.scalar_tensor_tensor(
                    out=g_t[:, j], in0=x1[:, j], scalar=vrow[:, j, 0:1],
                    in1=wb[:, j, 128:320], op0=_mult, op1=_mult)

            # e = t2b * y1b  (GPS, double-broadcast)
            e_t = wk.tile([P, J, MUL, 3], F32, name="e_t")
            nc.gpsimd.tensor_tensor(
                out=e_t, in0=t2.unsqueeze(3).broadcast_to([P, J, MUL, 3]),
                in1=y1b, op=_mult)

            # out1 = e + g  (DVE); out0 = m1y + md  (GPS)
            nc.vector.tensor_tensor(
                out=out_t[:, :, MUL:FEA],
                in0=e_t.rearrange("p j u d -> p j (u d)"), in1=g_t, op=_add)
            nc.gpsimd.tensor_tensor(out=out_t[:, :, 0:MUL], in0=m1y, in1=md,
                                    op=_add)

            # ---------- store ----------
            nc.sync.dma_start(out=out_v[m], in_=out_t)


def _build_program(flags):
    """flags = {key: (b1_nz, gbe_nz)}"""
    import contextlib
    nc = bacc.Bacc("TRN2", target_bir_lowering=False, debug=False)
    with tile.TileContext(nc) as tc:
        with contextlib.ExitStack() as ctx:
            glob = ctx.enter_context(tc.tile_pool(name="glob", bufs=1))
            ident = glob.tile([P, P], F32)
            make_identity(nc, ident)
            ident_bf = glob.tile([P, P], BF16)
            nc.scalar.copy(ident_bf, ident)
            magic8 = glob.tile([P, 2 * J], I32)
            nc.vector.memset(magic8, 0x5F3759DF)
            pools = (
                ctx.enter_context(tc.tile_pool(name="io", bufs=3)),
                ctx.enter_context(tc.tile_pool(name="wk", bufs=2)),
                ctx.enter_context(tc.tile_pool(name="st", bufs=2)),
                ctx.enter_context(tc.tile_pool(name="psmisc", bufs=2,
                                               space="PSUM")),
                ctx.enter_context(tc.tile_pool(name="psh", bufs=2, space="PSUM")),
                ctx.enter_context(tc.tile_pool(name="pswb", bufs=1, space="PSUM")),
            )
            for key in ("a", "b"):
                b1_nz, gbe_nz = flags[key]
                _build_key(nc, tc, ctx, key, b1_nz, gbe_nz, ident_bf, magic8, pools)
    nc.compile()
    return nc


def kernel(edge_fea_a, edge_vec_a, edge_len_a, W1_a, b1_a, g_a, be_a, W2_a,
           edge_fea_b, edge_vec_b, edge_len_b, W1_b, b1_b, g_b, be_b, W2_b):
    global last_exec_time_ns, last_results
    ins = {
        "a": (edge_fea_a, edge_vec_a, edge_len_a, W1_a, b1_a, g_a, be_a, W2_a),
        "b": (edge_fea_b, edge_vec_b, edge_len_b, W1_b, b1_b, g_b, be_b, W2_b),
    }
    prepped = {}
    flags = {}
    for key, (fea, vec, lng, W1, b1, g, be, W2) in ins.items():
        w1m, m2, w2big, b1_nz = _prep_weights(
            np.asarray(W1, np.float32), np.asarray(b1, np.float32),
            np.asarray(W2, np.float32))
        gbe_nz = bool(np.any(np.asarray(g) != 1.0) or np.any(np.asarray(be)))
        prepped[key] = (w1m, m2, w2big)
        flags[key] = (b1_nz, gbe_nz)

    ck = tuple(flags[k] for k in ("a", "b"))
    if ck not in _CACHE:
        _CACHE[ck] = _build_program(flags)
    nc = _CACHE[ck]

    in_maps = []
    for c in range(NCORE):
        sl = slice(c * ESH, (c + 1) * ESH)
        m = {}
        for key, (fea, vec, lng, W1, b1, g, be, W2) in ins.items():
            m[f"fea_{key}"] = np.ascontiguousarray(np.asarray(fea, np.float32)[sl])
            m[f"vec_{key}"] = np.ascontiguousarray(np.asarray(vec, np.float32)[sl])
            m[f"len_{key}"] = np.ascontiguousarray(np.asarray(lng, np.float32)[sl])
            m[f"w1m_{key}"] = prepped[key][0]
            m[f"m2_{key}"] = prepped[key][1]
            m[f"w2big_{key}"] = prepped[key][2]
            if flags[key][0]:
                m[f"b1_{key}"] = np.asarray(b1, np.float32)
                m[f"b1mu_{key}"] = np.full(
                    (P, 1), np.asarray(b1, np.float32).mean(), np.float32)
            if flags[key][1]:
                m[f"g_{key}"] = np.asarray(g, np.float32)
                m[f"be_{key}"] = np.asarray(be, np.float32)
        in_maps.append(m)

    trace = bool(int(os.environ.get("KERNEL_TRACE", "0")))
    res = run_bass_kernel_spmd(nc, in_maps, list(range(NCORE)), trace=trace)
    globals()["last_results"] = res
    last_exec_time_ns = res.exec_time_ns

    out_a = np.concatenate([np.asarray(res.results[c]["out_a"])
                            for c in range(NCORE)], axis=0)
    out_b = np.concatenate([np.asarray(res.results[c]["out_b"])
                            for c in range(NCORE)], axis=0)
    return (out_a, out_b)


# revision 15
# speedup vs baseline: 1.2216x; 1.0013x over previous
"""Trainium2 Bass kernel for DepthwiseTensorProductModuleDict.

Computes, for each key k in {a, b}:
    w = MLP(edge_len_k)           # Linear(64->128) -> LayerNorm -> silu -> Linear(128->256)
    out_k = DTP(edge_fea_k, edge_vec_k, w)   # depthwise uvu tensor product

Sharding: edge dimension split across 8 NeuronCores (pure data parallel),
both dict keys processed by every core on its edge shard. Weights replicated.

Layout: edges packed 4 per partition -> macro tiles of 512 edges
[128 partitions, 4 slots, features]. Per-macro pipeline:
  PE (bf16): 2x len transpose -> mm1 (N=128 + fused-mean N=2) ->
             4x a transpose -> mm2 (N=256 + N=128)
  ACT: len->bf16 cast, PSUM->SBUF copies, Square+accum (sum h^2),
       Sqrt (std), Silu(scale,bias) for layernorm+silu fusion
  DVE: fast-reciprocal (rstd), LN stats, PSUM-coupled DTP elementwise
  GPSIMD: SBUF-only DTP elementwise (contiguous APs only)
"""
import os
import numpy as np

import concourse.bass as bass
import concourse.tile as tile
from concourse import bacc, mybir
from concourse.bass_utils import run_bass_kernel_spmd
from concourse.masks import make_identity

F32 = mybir.dt.float32
BF16 = mybir.dt.bfloat16
I32 = mybir.dt.int32
P = 128          # partitions
J = 4            # edges per partition
MACRO = P * J    # 512 edges per macro tile
E = 131072       # total edges per key
NCORE = 8
ESH = E // NCORE          # 16384 edges per core per key
NM = ESH // MACRO         # 32 macros per key per core
MUL = 64
FEA = 256
RAD = 64
HID = 128
EPS = 1e-5

_mult = mybir.AluOpType.mult
_add = mybir.AluOpType.add
_sub = mybir.AluOpType.subtract

# cached compiled program (host-side) keyed by per-key flags
_CACHE = {}

last_exec_time_ns = None
last_results = None


def _prep_weights(W1, b1, W2):
    """Host-side weight packing (bf16 for the PE path).

    Returns:
      w1m   [64, 128] bf16: W1
      m2    [64, 132] bf16: [mu_col | 0 | W1 W1^T] split hi|lo (PSUM-accumulated
            for ~16-bit mantissa: mean + sum-h^2 quadratic form)
      w2big [128, 384] bf16: [w1 | w2 | w3rep | w4]  (pre-scaled)
      b1_nz flag
    """
    inv_s2 = np.float32(1.0 / np.sqrt(np.float32(2.0)))
    inv_s3 = np.float32(1.0 / np.sqrt(np.float32(3.0)))
    import ml_dtypes
    bf = ml_dtypes.bfloat16

    b1_nz = bool(np.any(b1))
    W1bf = W1.astype(bf).astype(np.float32)          # round first for consistency
    w1m = W1bf.astype(bf)                            # [64, 128]
    mu_col = W1bf.mean(axis=1, keepdims=True)        # [64, 1]
    pad = np.zeros_like(mu_col)
    M = W1bf @ W1bf.T                                # [64, 64] quadratic form
    m2f = np.hstack([mu_col, pad, M]).astype(np.float32)   # [64, 66]
    m2hi = m2f.astype(bf)
    m2lo = (m2f - m2hi.astype(np.float32)).astype(bf)
    m2 = np.hstack([m2hi, m2lo])                           # [64, 132] bf16

    w1 = W2[:, 0:64] * inv_s2
    w2 = W2[:, 64:128] * inv_s2
    w3 = W2[:, 128:192] * inv_s2
    w4 = W2[:, 192:256] * (inv_s2 * inv_s3)
    w3rep = np.repeat(w3, 3, axis=1)                       # [128, 192]
    w2big = np.concatenate([w1, w2, w3rep, w4], axis=1).astype(bf)  # [128, 384]
    return w1m, m2, w2big, b1_nz


def _build_key(nc, tc, ctx, key, b1_nz, gbe_nz, ident_bf, magic8, pools):
    """Emit instructions for one dict key's full shard (NM macros)."""
    fea = nc.dram_tensor(f"fea_{key}", [ESH, FEA], F32, kind="ExternalInput").ap()
    vec = nc.dram_tensor(f"vec_{key}", [ESH, 4], F32, kind="ExternalInput").ap()
    lng = nc.dram_tensor(f"len_{key}", [ESH, RAD], F32, kind="ExternalInput").ap()
    w1m_d = nc.dram_tensor(f"w1m_{key}", [RAD, HID], BF16,
                           kind="ExternalInput").ap()
    m2_d = nc.dram_tensor(f"m2_{key}", [RAD, 132], BF16,
                          kind="ExternalInput").ap()
    w2big_d = nc.dram_tensor(f"w2big_{key}", [HID, 384], BF16,
                             kind="ExternalInput").ap()
    out = nc.dram_tensor(f"out_{key}", [ESH, FEA], F32, kind="ExternalOutput").ap()
    b1_d = g_d = be_d = None
    if b1_nz:
        b1_d = nc.dram_tensor(f"b1_{key}", [HID], F32, kind="ExternalInput").ap()
    if gbe_nz:
        g_d = nc.dram_tensor(f"g_{key}", [HID], F32, kind="ExternalInput").ap()
        be_d = nc.dram_tensor(f"be_{key}", [HID], F32, kind="ExternalInput").ap()

    fea_v = fea.rearrange("(m p j) f -> m p j f", p=P, j=J)
    len_v = lng.rearrange("(m p j) f -> m p (j f)", p=P, j=J)   # [NM, 128, 256]
    out_v = out.rearrange("(m p j) f -> m p j f", p=P, j=J)
    vec_v = vec.rearrange("(m p j) f -> p m (j f)", p=P, j=J)   # [128, NM, 16]

    const = ctx.enter_context(tc.tile_pool(name=f"const_{key}", bufs=1))

    # --- weights ---
    w1m_sb = const.tile([RAD, HID], BF16)
    nc.sync.dma_start(out=w1m_sb, in_=w1m_d)
    m2_sb = const.tile([RAD, 132], BF16)
    nc.sync.dma_start(out=m2_sb, in_=m2_d)
    w2big_sb = const.tile([HID, 384], BF16)
    nc.sync.dma_start(out=w2big_sb, in_=w2big_d)

    b1rep = b1mu = grep = berep = None
    if b1_nz:
        b1mu_d = nc.dram_tensor(f"b1mu_{key}", [P, 1], F32,
                                kind="ExternalInput").ap()
        b1rep = const.tile([P, HID], F32)
        nc.sync.dma_start(out=b1rep, in_=b1_d.partition_broadcast(P))
        b1mu = const.tile([P, 1], F32)
        nc.sync.dma_start(out=b1mu, in_=b1mu_d)
    if gbe_nz:
        grep = const.tile([P, HID], F32)
        berep = const.tile([P, HID], F32)
        nc.sync.dma_start(out=grep, in_=g_d.partition_broadcast(P))
        nc.sync.dma_start(out=berep, in_=be_d.partition_broadcast(P))

    # --- whole-shard vec resident in SBUF ---
    vec_sb = const.tile([P, NM, J * 4], F32)
    nc.sync.dma_start(out=vec_sb, in_=vec_v)

    io, wk, st, ps_misc, ps_h, ps_wb = pools

    PAIR = 2
    for mp in range(NM // PAIR):
        ssq_p = st.tile([P, PAIR, J], F32, name="ssq_p")
        mus_p = st.tile([P, PAIR, J], F32, name="mus_p")
        saved = []
        for pm in range(PAIR):
            m = mp * PAIR + pm
            # ---------- loads ----------
            len_t = io.tile([P, J * RAD], F32, name="len_t")
            nc.sync.dma_start(out=len_t, in_=len_v[m])
            fea_t = io.tile([P, J, FEA], F32, name="fea_t")
            nc.sync.dma_start(out=fea_t, in_=fea_v[m])
            vrow = vec_sb[:, m, :].rearrange("p (j f) -> p j f", f=4)

            # ---------- len -> bf16, PE transpose, mm1 + [mu|M] ----------
            len_bf = wk.tile([P, J * RAD], BF16, name="len_bf")
            nc.scalar.copy(len_bf, len_t)
            lt_ps = ps_misc.tile([RAD, J * P], BF16, name="lt_ps", tag="misc")
            for j in range(J):
                nc.tensor.transpose(lt_ps[:, j * P:(j + 1) * P],
                                    len_bf[:, j * RAD:(j + 1) * RAD], ident_bf)
            lt_sb = wk.tile([RAD, J * P], BF16, name="lt_sb")
            nc.scalar.copy(lt_sb, lt_ps)

            h_ps = ps_h.tile([P, J, HID], F32, name="h_ps")
            mu_ps = ps_misc.tile([P, J, 66], F32, name="mu_ps", tag="misc")
            for j in range(J):
                slab = lt_sb[:, j * P:(j + 1) * P]
                nc.tensor.matmul(h_ps[:, j, :], slab, w1m_sb,
                                 start=True, stop=True)
                nc.tensor.matmul(mu_ps[:, j, :], slab, m2_sb[:, 0:66],
                                 start=True, stop=False)
                nc.tensor.matmul(mu_ps[:, j, :], slab, m2_sb[:, 66:132],
                                 start=False, stop=True)
            if b1_nz:
                hb = wk.tile([P, J, HID], F32, name="hb")
                nc.vector.tensor_tensor(
                    out=hb, in0=h_ps,
                    in1=b1rep.unsqueeze(1).broadcast_to([P, J, HID]), op=_add)
                h_src = hb
            else:
                h_src = h_ps

            # ssq = sum_i h_i^2 = sum_l (len @ M) * len   (quadratic form)
            qprod = wk.tile([P, J, RAD], F32, name="qprod")
            nc.vector.tensor_tensor(
                out=qprod, in0=mu_ps[:, :, 2:66],
                in1=len_bf.rearrange("p (j r) -> p j r", r=RAD), op=_mult)
            nc.vector.tensor_reduce(ssq_p[:, pm, :], qprod,
                                    axis=mybir.AxisListType.X,
                                    op=_add)
            nc.vector.tensor_copy(mus_p[:, pm, :], mu_ps[:, :, 0:1].squeeze(2))
            saved.append((m, fea_t, h_src, vrow))

        # ---------- layernorm stats (batched across the pair) ----------
        if b1_nz:
            musb = st.tile([P, PAIR, J], F32, name="musb")
            nc.vector.tensor_tensor(out=musb, in0=mus_p,
                                    in1=b1mu.broadcast_to([P, PAIR, J]),
                                    op=_add)
            mus_p = musb
        musq = st.tile([P, PAIR, J], F32, name="musq")
        nc.gpsimd.tensor_tensor(out=musq, in0=mus_p, in1=mus_p, op=_mult)
        var = st.tile([P, PAIR, J], F32, name="var")
        nc.vector.scalar_tensor_tensor(out=var, in0=ssq_p, scalar=1.0 / HID,
                                       in1=musq, op0=_mult, op1=_sub)
        # rstd via 1 Newton iteration from the fast-inverse-sqrt seed
        vpe = st.tile([P, PAIR, J], F32, name="vpe")
        nc.vector.tensor_scalar(out=vpe, in0=var, scalar1=EPS, scalar2=None,
                                op0=_add)
        nvpe = st.tile([P, PAIR, J], F32, name="nvpe")
        nc.vector.tensor_scalar(out=nvpe, in0=var, scalar1=-0.5,
                                scalar2=-EPS / 2, op0=_mult, op1=_add)
        ibits = st.tile([P, PAIR, J], I32, name="ibits")
        nc.vector.tensor_scalar(out=ibits, in0=vpe.bitcast(I32), scalar1=1,
                                scalar2=None,
                                op0=mybir.AluOpType.logical_shift_right)
        seed = st.tile([P, PAIR, J], I32, name="seed")
        nc.vector.tensor_tensor(out=seed,
                                in0=magic8.rearrange("p (a j) -> p a j", a=2),
                                in1=ibits, op=_sub)
        y2 = st.tile([P, PAIR, J], F32, name="y2")
        nc.gpsimd.tensor_tensor(out=y2, in0=seed.bitcast(F32),
                                in1=seed.bitcast(F32), op=_mult)
        w_ = st.tile([P, PAIR, J], F32, name="w_")
        nc.gpsimd.tensor_tensor(out=w_, in0=y2, in1=nvpe, op=_mult)
        y_a = st.tile([P, PAIR, J], F32, name="y_a")
        nc.vector.scalar_tensor_tensor(out=y_a, in0=w_, scalar=1.5,
                                       in1=seed.bitcast(F32), op0=_add,
                                       op1=_mult)
        # second Newton iteration (cheap on GPS, keeps rstd ~1e-6)
        y2b = st.tile([P, PAIR, J], F32, name="y2b")
        nc.gpsimd.tensor_tensor(out=y2b, in0=y_a, in1=y_a, op=_mult)
        w2b_ = st.tile([P, PAIR, J], F32, name="w2b_")
        nc.gpsimd.tensor_tensor(out=w2b_, in0=y2b, in1=nvpe, op=_mult)
        rstd = st.tile([P, PAIR, J], F32, name="rstd")
        nc.vector.scalar_tensor_tensor(out=rstd, in0=w2b_, scalar=1.5,
                                       in1=y_a, op0=_add, op1=_mult)
        nbias = st.tile([P, PAIR, J], F32, name="nbias")
        nc.vector.scalar_tensor_tensor(out=nbias, in0=mus_p, scalar=-1.0,
                                       in1=rstd, op0=_mult, op1=_mult)

        for pm in range(PAIR):
            m, fea_t, h_src, vrow = saved[pm]
            # ---------- normalize + silu -> bf16 a ----------
            a_sb = wk.tile([P, J, HID], BF16, name="a_sb")
            if not gbe_nz:
                for j in range(J):
                    nc.scalar.activation(a_sb[:, j], h_src[:, j, :],
                                         mybir.ActivationFunctionType.Silu,
                                         bias=nbias[:, pm, j:j + 1],
                                         scale=rstd[:, pm, j:j + 1])
            else:
                hn = wk.tile([P, J, HID], F32, name="hn")
                for j in range(J):
                    nc.scalar.activation(hn[:, j], h_src[:, j, :],
                                         mybir.ActivationFunctionType.Identity,
                                         bias=nbias[:, pm, j:j + 1],
                                         scale=rstd[:, pm, j:j + 1])
                hg = wk.tile([P, J, HID], F32, name="hg")
                nc.vector.tensor_tensor(
                    out=hg, in0=hn,
                    in1=grep.unsqueeze(1).broadcast_to([P, J, HID]), op=_mult)
                nc.vector.tensor_tensor(
                    out=hg, in0=hg,
                    in1=berep.unsqueeze(1).broadcast_to([P, J, HID]), op=_add)
                for j in range(J):
                    nc.scalar.activation(a_sb[:, j], hg[:, j],
                                         mybir.ActivationFunctionType.Silu)

            # ---------- PE: transpose a, mm2 (single N=384) ----------
            at_ps = ps_misc.tile([P, J, HID], BF16, name="at_ps", tag="misc")
            for j in range(J):
                nc.tensor.transpose(at_ps[:, j, :], a_sb[:, j, :], ident_bf)
            at_sb = wk.tile([P, J, HID], BF16, name="at_sb")
            nc.scalar.copy(at_sb, at_ps)

            wb = ps_wb.tile([P, J, 384], F32, name="wb",
                            padded_shape=[P, J, 512])
            for j in range(J):
                nc.tensor.matmul(wb[:, j, :], at_sb[:, j, :], w2big_sb,
                                 start=True, stop=True)

            # ---------- DTP ----------
            out_t = io.tile([P, J, FEA], F32, name="out_t")
            x0 = fea_t[:, :, 0:MUL]                    # [P,J,64]
            x1 = fea_t[:, :, MUL:FEA]                  # [P,J,192]
            x1v = x1.rearrange("p j (u d) -> p j u d", d=3)
            y1b = vrow[:, :, 1:4].unsqueeze(2).broadcast_to([P, J, MUL, 3])

            # b = x1 * y1b  (GPS)
            b_t = wk.tile([P, J, MUL, 3], F32, name="b_t")
            nc.gpsimd.tensor_tensor(out=b_t, in0=x1v, in1=y1b, op=_mult)
            # d = sum_d b   (strided adds: DVE + GPS)
            d_t = wk.tile([P, J, MUL], F32, name="d_t")
            nc.vector.tensor_tensor(out=d_t, in0=b_t[:, :, :, 0],
                                    in1=b_t[:, :, :, 1], op=_add)
            d2_t = wk.tile([P, J, MUL], F32, name="d2_t")
            nc.gpsimd.tensor_tensor(out=d2_t, in0=d_t, in1=b_t[:, :, :, 2],
                                    op=_add)

            # t2 = w2' * x0   (DVE, PSUM in0)
            t2 = wk.tile([P, J, MUL], F32, name="t2")
            nc.vector.tensor_tensor(out=t2, in0=wb[:, :, 64:128], in1=x0,
                                    op=_mult)
            # m1y = (x0*y0)*w1'  (DVE STT per j)
            m1y = wk.tile([P, J, MUL], F32, name="m1y")
            for j in range(J):
                nc.vector.scalar_tensor_tensor(
                    out=m1y[:, j], in0=x0[:, j], scalar=vrow[:, j, 0:1],
                    in1=wb[:, j, 0:64], op0=_mult, op1=_mult)
            # md = d * w4'  (DVE, PSUM in1)
            md = wk.tile([P, J, MUL], F32, name="md")
            nc.vector.tensor_tensor(out=md, in0=d2_t, in1=wb[:, :, 320:384],
                                    op=_mult)
            # g = (x1 * y0) * w3rep'  (DVE STT per j; wb is PSUM)
            g_t = wk.tile([P, J, MUL * 3], F32, name="g_t")
            for j in range(J):
                nc.vector.scalar_tensor_tensor(
                    out=g_t[:, j], in0=x1[:, j], scalar=vrow[:, j, 0:1],
                    in1=wb[:, j, 128:320], op0=_mult, op1=_mult)

            # e = t2b * y1b  (GPS, double-broadcast)
            e_t = wk.tile([P, J, MUL, 3], F32, name="e_t")
            nc.gpsimd.tensor_tensor(
                out=e_t, in0=t2.unsqueeze(3).broadcast_to([P, J, MUL, 3]),
                in1=y1b, op=_mult)

            # out1 = e + g  (DVE); out0 = m1y + md  (GPS)
            nc.vector.tensor_tensor(
                out=out_t[:, :, MUL:FEA],
                in0=e_t.rearrange("p j u d -> p j (u d)"), in1=g_t, op=_add)
            nc.gpsimd.tensor_tensor(out=out_t[:, :, 0:MUL], in0=m1y, in1=md,
                                    op=_add)

            # ---------- store ----------
            nc.sync.dma_start(out=out_v[m], in_=out_t)


def _build_program(flags):
    """flags = {key: (b1_nz, gbe_nz)}"""
    import contextlib
    nc = bacc.Bacc("TRN2", target_bir_lowering=False, debug=False)
    with tile.TileContext(nc) as tc:
        with contextlib.ExitStack() as ctx:
            glob = ctx.enter_context(tc.tile_pool(name="glob", bufs=1))
            ident = glob.tile([P, P], F32)
            make_identity(nc, ident)
            ident_bf = glob.tile([P, P], BF16)
            nc.scalar.copy(ident_bf, ident)
            magic8 = glob.tile([P, 2 * J], I32)
            nc.vector.memset(magic8, 0x5F3759DF)
            pools = (
                ctx.enter_context(tc.tile_pool(name="io", bufs=3)),
                ctx.enter_context(tc.tile_pool(name="wk", bufs=2)),
                ctx.enter_context(tc.tile_pool(name="st", bufs=2)),
                ctx.enter_context(tc.tile_pool(name="psmisc", bufs=2,
                                               space="PSUM")),
                ctx.enter_context(tc.tile_pool(name="psh", bufs=2, space="PSUM")),
                ctx.enter_context(tc.tile_pool(name="pswb", bufs=1, space="PSUM")),
            )
            for key in ("a", "b"):
                b1_nz, gbe_nz = flags[key]
                _build_key(nc, tc, ctx, key, b1_nz, gbe_nz, ident_bf, magic8, pools)
    nc.compile()
    return nc


def kernel(edge_fea_a, edge_vec_a, edge_len_a, W1_a, b1_a, g_a, be_a, W2_a,
           edge_fea_b, edge_vec_b, edge_len_b, W1_b, b1_b, g_b, be_b, W2_b):
    global last_exec_time_ns, last_results
    ins = {
        "a": (edge_fea_a, edge_vec_a, edge_len_a, W1_a, b1_a, g_a, be_a, W2_a),
        "b": (edge_fea_b, edge_vec_b, edge_len_b, W1_b, b1_b, g_b, be_b, W2_b),
    }
    prepped = {}
    flags = {}
    for key, (fea, vec, lng, W1, b1, g, be, W2) in ins.items():
        w1m, m2, w2big, b1_nz = _prep_weights(
            np.asarray(W1, np.float32), np.asarray(b1, np.float32),
            np.asarray(W2, np.float32))
        gbe_nz = bool(np.any(np.asarray(g) != 1.0) or np.any(np.asarray(be)))
        prepped[key] = (w1m, m2, w2big)
        flags[key] = (b1_nz, gbe_nz)

    ck = tuple(flags[k] for k in ("a", "b"))
    if ck not in _CACHE:
        _CACHE[ck] = _build_program(flags)
    nc = _CACHE[ck]

    in_maps = []
    for c in range(NCORE):
        sl = slice(c * ESH, (c + 1) * ESH)
        m = {}
        for key, (fea, vec, lng, W1, b1, g, be, W2) in ins.items():
            m[f"fea_{key}"] = np.ascontiguousarray(np.asarray(fea, np.float32)[sl])
            m[f"vec_{key}"] = np.ascontiguousarray(np.asarray(vec, np.float32)[sl])
            m[f"len_{key}"] = np.ascontiguousarray(np.asarray(lng, np.float32)[sl])
            m[f"w1m_{key}"] = prepped[key][0]
            m[f"m2_{key}"] = prepped[key][1]
            m[f"w2big_{key}"] = prepped[key][2]
            if flags[key][0]:
                m[f"b1_{key}"] = np.asarray(b1, np.float32)
                m[f"b1mu_{key}"] = np.full(
                    (P, 1), np.asarray(b1, np.float32).mean(), np.float32)
            if flags[key][1]:
                m[f"g_{key}"] = np.asarray(g, np.float32)
                m[f"be_{key}"] = np.asarray(be, np.float32)
        in_maps.append(m)

    trace = bool(int(os.environ.get("KERNEL_TRACE", "0")))
    res = run_bass_kernel_spmd(nc, in_maps, list(range(NCORE)), trace=trace)
    globals()["last_results"] = res
    last_exec_time_ns = res.exec_time_ns

    out_a = np.concatenate([np.asarray(res.results[c]["out_a"])
                            for c in range(NCORE)], axis=0)
    out_b = np.concatenate([np.asarray(res.results[c]["out_b"])
                            for c in range(NCORE)], axis=0)
    return (out_a, out_b)


# revision 16
# speedup vs baseline: 1.2906x; 1.0565x over previous
"""Trainium2 Bass kernel for DepthwiseTensorProductModuleDict.

Computes, for each key k in {a, b}:
    w = MLP(edge_len_k)           # Linear(64->128) -> LayerNorm -> silu -> Linear(128->256)
    out_k = DTP(edge_fea_k, edge_vec_k, w)   # depthwise uvu tensor product

Sharding: edge dimension split across 8 NeuronCores (pure data parallel),
both dict keys processed by every core on its edge shard. Weights replicated.

Layout: edges packed 4 per partition -> macro tiles of 512 edges
[128 partitions, 4 slots, features]. Per-macro pipeline:
  PE (bf16): 2x len transpose -> mm1 (N=128 + fused-mean N=2) ->
             4x a transpose -> mm2 (N=256 + N=128)
  ACT: len->bf16 cast, PSUM->SBUF copies, Square+accum (sum h^2),
       Sqrt (std), Silu(scale,bias) for layernorm+silu fusion
  DVE: fast-reciprocal (rstd), LN stats, PSUM-coupled DTP elementwise
  GPSIMD: SBUF-only DTP elementwise (contiguous APs only)
"""
import os
import numpy as np

import concourse.bass as bass
import concourse.tile as tile
from concourse import bacc, mybir
from concourse.bass_utils import run_bass_kernel_spmd
from concourse.masks import make_identity

F32 = mybir.dt.float32
BF16 = mybir.dt.bfloat16
I32 = mybir.dt.int32
P = 128          # partitions
J = 4            # edges per partition
MACRO = P * J    # 512 edges per macro tile
E = 131072       # total edges per key
NCORE = 8
ESH = E // NCORE          # 16384 edges per core per key
NM = ESH // MACRO         # 32 macros per key per core
MUL = 64
FEA = 256
RAD = 64
HID = 128
EPS = 1e-5

_mult = mybir.AluOpType.mult
_add = mybir.AluOpType.add
_sub = mybir.AluOpType.subtract

# cached compiled program (host-side) keyed by per-key flags
_CACHE = {}

last_exec_time_ns = None
last_results = None


def _prep_weights(W1, b1, W2):
    """Host-side weight packing (bf16 for the PE path).

    Returns:
      w1m   [64, 128] bf16: W1
      m2    [64, 132] bf16: [mu_col | 0 | W1 W1^T] split hi|lo (PSUM-accumulated
            for ~16-bit mantissa: mean + sum-h^2 quadratic form)
      w2big [128, 384] bf16: [w1 | w2 | w3rep | w4]  (pre-scaled)
      b1_nz flag
    """
    inv_s2 = np.float32(1.0 / np.sqrt(np.float32(2.0)))
    inv_s3 = np.float32(1.0 / np.sqrt(np.float32(3.0)))
    import ml_dtypes
    bf = ml_dtypes.bfloat16

    b1_nz = bool(np.any(b1))
    W1bf = W1.astype(bf).astype(np.float32)          # round first for consistency
    w1m = W1bf.astype(bf)                            # [64, 128]
    mu_col = W1bf.mean(axis=1, keepdims=True)        # [64, 1]
    pad = np.zeros_like(mu_col)
    M = W1bf @ W1bf.T                                # [64, 64] quadratic form
    m2f = np.hstack([mu_col, pad, M]).astype(np.float32)   # [64, 66]
    m2hi = m2f.astype(bf)
    m2lo = (m2f - m2hi.astype(np.float32)).astype(bf)
    m2 = np.hstack([m2hi, m2lo])                           # [64, 132] bf16

    w1 = W2[:, 0:64] * inv_s2
    w2 = W2[:, 64:128] * inv_s2
    w3 = W2[:, 128:192] * inv_s2
    w4 = W2[:, 192:256] * (inv_s2 * inv_s3)
    w3rep = np.repeat(w3, 3, axis=1)                       # [128, 192]
    w2big = np.concatenate([w1, w2, w3rep, w4], axis=1).astype(bf)  # [128, 384]
    return w1m, m2, w2big, b1_nz


def _build_key(nc, tc, ctx, key, b1_nz, gbe_nz, ident_bf, magic8,
               eps_t, neps2_t, pools):
    """Emit instructions for one dict key's full shard (NM macros)."""
    fea = nc.dram_tensor(f"fea_{key}", [ESH, FEA], F32, kind="ExternalInput").ap()
    vec = nc.dram_tensor(f"vec_{key}", [ESH, 4], F32, kind="ExternalInput").ap()
    lng = nc.dram_tensor(f"len_{key}", [ESH, RAD], F32, kind="ExternalInput").ap()
    w1m_d = nc.dram_tensor(f"w1m_{key}", [RAD, HID], BF16,
                           kind="ExternalInput").ap()
    m2_d = nc.dram_tensor(f"m2_{key}", [RAD, 132], BF16,
                          kind="ExternalInput").ap()
    w2big_d = nc.dram_tensor(f"w2big_{key}", [HID, 384], BF16,
                             kind="ExternalInput").ap()
    out = nc.dram_tensor(f"out_{key}", [ESH, FEA], F32, kind="ExternalOutput").ap()
    b1_d = g_d = be_d = None
    if b1_nz:
        b1_d = nc.dram_tensor(f"b1_{key}", [HID], F32, kind="ExternalInput").ap()
    if gbe_nz:
        g_d = nc.dram_tensor(f"g_{key}", [HID], F32, kind="ExternalInput").ap()
        be_d = nc.dram_tensor(f"be_{key}", [HID], F32, kind="ExternalInput").ap()

    fea_v = fea.rearrange("(m p j) f -> m p j f", p=P, j=J)
    len_v = lng.rearrange("(m p j) f -> m p (j f)", p=P, j=J)   # [NM, 128, 256]
    out_v = out.rearrange("(m p j) f -> m p j f", p=P, j=J)
    vec_v = vec.rearrange("(m p j) f -> p m (j f)", p=P, j=J)   # [128, NM, 16]

    const = ctx.enter_context(tc.tile_pool(name=f"const_{key}", bufs=1))

    # --- weights ---
    w1m_sb = const.tile([RAD, HID], BF16)
    nc.sync.dma_start(out=w1m_sb, in_=w1m_d)
    m2_sb = const.tile([RAD, 132], BF16)
    nc.sync.dma_start(out=m2_sb, in_=m2_d)
    w2big_sb = const.tile([HID, 384], BF16)
    nc.sync.dma_start(out=w2big_sb, in_=w2big_d)

    b1rep = b1mu = grep = berep = None
    if b1_nz:
        b1mu_d = nc.dram_tensor(f"b1mu_{key}", [P, 1], F32,
                                kind="ExternalInput").ap()
        b1rep = const.tile([P, HID], F32)
        nc.sync.dma_start(out=b1rep, in_=b1_d.partition_broadcast(P))
        b1mu = const.tile([P, 1], F32)
        nc.sync.dma_start(out=b1mu, in_=b1mu_d)
    if gbe_nz:
        grep = const.tile([P, HID], F32)
        berep = const.tile([P, HID], F32)
        nc.sync.dma_start(out=grep, in_=g_d.partition_broadcast(P))
        nc.sync.dma_start(out=berep, in_=be_d.partition_broadcast(P))

    # --- whole-shard vec resident in SBUF ---
    vec_sb = const.tile([P, NM, J * 4], F32)
    nc.sync.dma_start(out=vec_sb, in_=vec_v)

    io, wk, st, ps_misc, ps_h, ps_wb = pools

    PAIR = 2
    for mp in range(NM // PAIR):
        ssq_p = st.tile([P, PAIR, J], F32, name="ssq_p")
        mus_p = st.tile([P, PAIR, J], F32, name="mus_p")
        saved = []
        for pm in range(PAIR):
            m = mp * PAIR + pm
            # ---------- loads ----------
            len_t = io.tile([P, J * RAD], F32, name="len_t")
            nc.sync.dma_start(out=len_t, in_=len_v[m])
            fea_t = io.tile([P, J, FEA], F32, name="fea_t")
            nc.sync.dma_start(out=fea_t, in_=fea_v[m])
            vrow = vec_sb[:, m, :].rearrange("p (j f) -> p j f", f=4)

            # ---------- len -> bf16, PE transpose, mm1 + [mu|M] ----------
            len_bf = wk.tile([P, J * RAD], BF16, name="len_bf")
            nc.scalar.copy(len_bf, len_t)
            lt_ps = ps_misc.tile([RAD, J * P], BF16, name="lt_ps", tag="misc")
            for j in range(J):
                nc.tensor.transpose(lt_ps[:, j * P:(j + 1) * P],
                                    len_bf[:, j * RAD:(j + 1) * RAD], ident_bf)
            lt_sb = wk.tile([RAD, J * P], BF16, name="lt_sb")
            nc.scalar.copy(lt_sb, lt_ps)

            h_ps = ps_h.tile([P, J, HID], F32, name="h_ps")
            mu_ps = ps_misc.tile([P, J, 66], F32, name="mu_ps", tag="misc")
            for j in range(J):
                slab = lt_sb[:, j * P:(j + 1) * P]
                nc.tensor.matmul(h_ps[:, j, :], slab, w1m_sb,
                                 start=True, stop=True)
                nc.tensor.matmul(mu_ps[:, j, :], slab, m2_sb[:, 0:66],
                                 start=True, stop=False)
                nc.tensor.matmul(mu_ps[:, j, :], slab, m2_sb[:, 66:132],
                                 start=False, stop=True)
            if b1_nz:
                hb = wk.tile([P, J, HID], F32, name="hb")
                nc.vector.tensor_tensor(
                    out=hb, in0=h_ps,
                    in1=b1rep.unsqueeze(1).broadcast_to([P, J, HID]), op=_add)
                h_src = hb
            else:
                h_src = h_ps

            # ssq = sum_i h_i^2 = sum_l (len @ M) * len   (quadratic form)
            qprod = wk.tile([P, J, RAD], F32, name="qprod")
            nc.vector.tensor_tensor(
                out=qprod, in0=mu_ps[:, :, 2:66],
                in1=len_bf.rearrange("p (j r) -> p j r", r=RAD), op=_mult)
            nc.vector.tensor_reduce(ssq_p[:, pm, :], qprod,
                                    axis=mybir.AxisListType.X,
                                    op=_add)
            nc.scalar.copy(mus_p[:, pm, :], mu_ps[:, :, 0:1].squeeze(2))
            saved.append((m, fea_t, h_src, vrow))

        # ---------- layernorm stats (batched across the pair) ----------
        if b1_nz:
            musb = st.tile([P, PAIR, J], F32, name="musb")
            nc.vector.tensor_tensor(out=musb, in0=mus_p,
                                    in1=b1mu.broadcast_to([P, PAIR, J]),
                                    op=_add)
            mus_p = musb
        musq = st.tile([P, PAIR, J], F32, name="musq")
        nc.scalar.activation(musq, mus_p, mybir.ActivationFunctionType.Square)
        var = st.tile([P, PAIR, J], F32, name="var")
        nc.vector.scalar_tensor_tensor(out=var, in0=ssq_p, scalar=1.0 / HID,
                                       in1=musq, op0=_mult, op1=_sub)
        # rstd via 1 Newton iteration from the fast-inverse-sqrt seed
        vpe = st.tile([P, PAIR, J], F32, name="vpe")
        nc.scalar.activation(vpe, var, mybir.ActivationFunctionType.Identity,
                             bias=eps_t[:, 0:1])
        nvpe = st.tile([P, PAIR, J], F32, name="nvpe")
        nc.scalar.activation(nvpe, var, mybir.ActivationFunctionType.Identity,
                             bias=neps2_t[:, 0:1], scale=-0.5)
        ibits = st.tile([P, PAIR, J], I32, name="ibits")
        nc.vector.tensor_scalar(out=ibits, in0=vpe.bitcast(I32), scalar1=1,
                                scalar2=None,
                                op0=mybir.AluOpType.logical_shift_right)
        seed = st.tile([P, PAIR, J], I32, name="seed")
        nc.vector.tensor_tensor(out=seed,
                                in0=magic8.rearrange("p (a j) -> p a j", a=2),
                                in1=ibits, op=_sub)
        y2 = st.tile([P, PAIR, J], F32, name="y2")
        nc.gpsimd.tensor_tensor(out=y2, in0=seed.bitcast(F32),
                                in1=seed.bitcast(F32), op=_mult)
        w_ = st.tile([P, PAIR, J], F32, name="w_")
        nc.gpsimd.tensor_tensor(out=w_, in0=y2, in1=nvpe, op=_mult)
        y_a = st.tile([P, PAIR, J], F32, name="y_a")
        nc.vector.scalar_tensor_tensor(out=y_a, in0=w_, scalar=1.5,
                                       in1=seed.bitcast(F32), op0=_add,
                                       op1=_mult)
        # second Newton iteration (cheap on GPS, keeps rstd ~1e-6)
        y2b = st.tile([P, PAIR, J], F32, name="y2b")
        nc.gpsimd.tensor_tensor(out=y2b, in0=y_a, in1=y_a, op=_mult)
        w2b_ = st.tile([P, PAIR, J], F32, name="w2b_")
        nc.gpsimd.tensor_tensor(out=w2b_, in0=y2b, in1=nvpe, op=_mult)
        rstd = st.tile([P, PAIR, J], F32, name="rstd")
        nc.vector.scalar_tensor_tensor(out=rstd, in0=w2b_, scalar=1.5,
                                       in1=y_a, op0=_add, op1=_mult)
        nbias = st.tile([P, PAIR, J], F32, name="nbias")
        nc.vector.scalar_tensor_tensor(out=nbias, in0=mus_p, scalar=-1.0,
                                       in1=rstd, op0=_mult, op1=_mult)

        for pm in range(PAIR):
            m, fea_t, h_src, vrow = saved[pm]
            # ---------- normalize + silu -> bf16 a ----------
            a_sb = wk.tile([P, J, HID], BF16, name="a_sb")
            if not gbe_nz:
                for j in range(J):
                    nc.scalar.activation(a_sb[:, j], h_src[:, j, :],
                                         mybir.ActivationFunctionType.Silu,
                                         bias=nbias[:, pm, j:j + 1],
                                         scale=rstd[:, pm, j:j + 1])
            else:
                hn = wk.tile([P, J, HID], F32, name="hn")
                for j in range(J):
                    nc.scalar.activation(hn[:, j], h_src[:, j, :],
                                         mybir.ActivationFunctionType.Identity,
                                         bias=nbias[:, pm, j:j + 1],
                                         scale=rstd[:, pm, j:j + 1])
                hg = wk.tile([P, J, HID], F32, name="hg")
                nc.vector.tensor_tensor(
                    out=hg, in0=hn,
                    in1=grep.unsqueeze(1).broadcast_to([P, J, HID]), op=_mult)
                nc.vector.tensor_tensor(
                    out=hg, in0=hg,
                    in1=berep.unsqueeze(1).broadcast_to([P, J, HID]), op=_add)
                for j in range(J):
                    nc.scalar.activation(a_sb[:, j], hg[:, j],
                                         mybir.ActivationFunctionType.Silu)

            # ---------- PE: transpose a, mm2 (single N=384) ----------
            at_ps = ps_misc.tile([P, J, HID], BF16, name="at_ps", tag="misc")
            for j in range(J):
                nc.tensor.transpose(at_ps[:, j, :], a_sb[:, j, :], ident_bf)
            at_sb = wk.tile([P, J, HID], BF16, name="at_sb")
            nc.scalar.copy(at_sb, at_ps)

            wb = ps_wb.tile([P, J, 384], F32, name="wb",
                            padded_shape=[P, J, 512])
            for j in range(J):
                nc.tensor.matmul(wb[:, j, :], at_sb[:, j, :], w2big_sb,
                                 start=True, stop=True)

            # ---------- DTP ----------
            out_t = io.tile([P, J, FEA], F32, name="out_t")
            x0 = fea_t[:, :, 0:MUL]                    # [P,J,64]
            x1 = fea_t[:, :, MUL:FEA]                  # [P,J,192]
            x1v = x1.rearrange("p j (u d) -> p j u d", d=3)
            y1b = vrow[:, :, 1:4].unsqueeze(2).broadcast_to([P, J, MUL, 3])

            # b = x1 * y1b  (GPS)
            b_t = wk.tile([P, J, MUL, 3], F32, name="b_t")
            nc.gpsimd.tensor_tensor(out=b_t, in0=x1v, in1=y1b, op=_mult)
            # d = sum_d b   (strided adds: DVE + GPS)
            d_t = wk.tile([P, J, MUL], F32, name="d_t")
            nc.vector.tensor_tensor(out=d_t, in0=b_t[:, :, :, 0],
                                    in1=b_t[:, :, :, 1], op=_add)
            d2_t = wk.tile([P, J, MUL], F32, name="d2_t")
            nc.gpsimd.tensor_tensor(out=d2_t, in0=d_t, in1=b_t[:, :, :, 2],
                                    op=_add)

            # t2 = w2' * x0   (DVE, PSUM in0)
            t2 = wk.tile([P, J, MUL], F32, name="t2")
            nc.vector.tensor_tensor(out=t2, in0=wb[:, :, 64:128], in1=x0,
                                    op=_mult)
            # m1y = (x0*y0)*w1'  (GPS broadcast mult + batched DVE TT)
            p1 = wk.tile([P, J, MUL], F32, name="p1")
            nc.gpsimd.tensor_tensor(
                out=p1, in0=x0,
                in1=vrow[:, :, 0:1].broadcast_to([P, J, MUL]), op=_mult)
            m1y = wk.tile([P, J, MUL], F32, name="m1y")
            nc.vector.tensor_tensor(out=m1y, in0=p1, in1=wb[:, :, 0:64],
                                    op=_mult)
            # md = d * w4'  (DVE, PSUM in1)
            md = wk.tile([P, J, MUL], F32, name="md")
            nc.vector.tensor_tensor(out=md, in0=d2_t, in1=wb[:, :, 320:384],
                                    op=_mult)
            # g = (x1 * y0) * w3rep'  (DVE STT per j; wb is PSUM)
            g_t = wk.tile([P, J, MUL * 3], F32, name="g_t")
            for j in range(J):
                nc.vector.scalar_tensor_tensor(
                    out=g_t[:, j], in0=x1[:, j], scalar=vrow[:, j, 0:1],
                    in1=wb[:, j, 128:320], op0=_mult, op1=_mult)

            # e = t2b * y1b  (GPS, double-broadcast)
            e_t = wk.tile([P, J, MUL, 3], F32, name="e_t")
            nc.gpsimd.tensor_tensor(
                out=e_t, in0=t2.unsqueeze(3).broadcast_to([P, J, MUL, 3]),
                in1=y1b, op=_mult)

            # out1 = e + g  (DVE); out0 = m1y + md  (GPS)
            nc.vector.tensor_tensor(
                out=out_t[:, :, MUL:FEA],
                in0=e_t.rearrange("p j u d -> p j (u d)"), in1=g_t, op=_add)
            nc.gpsimd.tensor_tensor(out=out_t[:, :, 0:MUL], in0=m1y, in1=md,
                                    op=_add)

            # ---------- store ----------
            nc.sync.dma_start(out=out_v[m], in_=out_t)


def _build_program(flags):
    """flags = {key: (b1_nz, gbe_nz)}"""
    import contextlib
    nc = bacc.Bacc("TRN2", target_bir_lowering=False, debug=False)
    with tile.TileContext(nc) as tc:
        with contextlib.ExitStack() as ctx:
            glob = ctx.enter_context(tc.tile_pool(name="glob", bufs=1))
            ident = glob.tile([P, P], F32)
            make_identity(nc, ident)
            ident_bf = glob.tile([P, P], BF16)
            nc.scalar.copy(ident_bf, ident)
            magic8 = glob.tile([P, 2 * J], I32)
            nc.vector.memset(magic8, 0x5F3759DF)
            eps_t = glob.tile([P, 1], F32)
            nc.vector.memset(eps_t, EPS)
            neps2_t = glob.tile([P, 1], F32)
            nc.vector.memset(neps2_t, -EPS / 2)
            pools = (
                ctx.enter_context(tc.tile_pool(name="io", bufs=3)),
                ctx.enter_context(tc.tile_pool(name="wk", bufs=2)),
                ctx.enter_context(tc.tile_pool(name="st", bufs=2)),
                ctx.enter_context(tc.tile_pool(name="psmisc", bufs=2,
                                               space="PSUM")),
                ctx.enter_context(tc.tile_pool(name="psh", bufs=2, space="PSUM")),
                ctx.enter_context(tc.tile_pool(name="pswb", bufs=1, space="PSUM")),
            )
            for key in ("a", "b"):
                b1_nz, gbe_nz = flags[key]
                _build_key(nc, tc, ctx, key, b1_nz, gbe_nz, ident_bf, magic8,
                           eps_t, neps2_t, pools)
    nc.compile()
    return nc


def kernel(edge_fea_a, edge_vec_a, edge_len_a, W1_a, b1_a, g_a, be_a, W2_a,
           edge_fea_b, edge_vec_b, edge_len_b, W1_b, b1_b, g_b, be_b, W2_b):
    global last_exec_time_ns, last_results
    ins = {
        "a": (edge_fea_a, edge_vec_a, edge_len_a, W1_a, b1_a, g_a, be_a, W2_a),
        "b": (edge_fea_b, edge_vec_b, edge_len_b, W1_b, b1_b, g_b, be_b, W2_b),
    }
    prepped = {}
    flags = {}
    for key, (fea, vec, lng, W1, b1, g, be, W2) in ins.items():
        w1m, m2, w2big, b1_nz = _prep_weights(
            np.asarray(W1, np.float32), np.asarray(b1, np.float32),
            np.asarray(W2, np.float32))
        gbe_nz = bool(np.any(np.asarray(g) != 1.0) or np.any(np.asarray(be)))
        prepped[key] = (w1m, m2, w2big)
        flags[key] = (b1_nz, gbe_nz)

    ck = tuple(flags[k] for k in ("a", "b"))
    if ck not in _CACHE:
        _CACHE[ck] = _build_program(flags)
    nc = _CACHE[ck]

    in_maps = []
    for c in range(NCORE):
        sl = slice(c * ESH, (c + 1) * ESH)
        m = {}
        for key, (fea, vec, lng, W1, b1, g, be, W2) in ins.items():
            m[f"fea_{key}"] = np.ascontiguousarray(np.asarray(fea, np.float32)[sl])
            m[f"vec_{key}"] = np.ascontiguousarray(np.asarray(vec, np.float32)[sl])
            m[f"len_{key}"] = np.ascontiguousarray(np.asarray(lng, np.float32)[sl])
            m[f"w1m_{key}"] = prepped[key][0]
            m[f"m2_{key}"] = prepped[key][1]
            m[f"w2big_{key}"] = prepped[key][2]
            if flags[key][0]:
                m[f"b1_{key}"] = np.asarray(b1, np.float32)
                m[f"b1mu_{key}"] = np.full(
                    (P, 1), np.asarray(b1, np.float32).mean(), np.float32)
            if flags[key][1]:
                m[f"g_{key}"] = np.asarray(g, np.float32)
                m[f"be_{key}"] = np.asarray(be, np.float32)
        in_maps.append(m)

    trace = bool(int(os.environ.get("KERNEL_TRACE", "0")))
    res = run_bass_kernel_spmd(nc, in_maps, list(range(NCORE)), trace=trace)
    globals()["last_results"] = res
    last_exec_time_ns = res.exec_time_ns

    out_a = np.concatenate([np.asarray(res.results[c]["out_a"])
                            for c in range(NCORE)], axis=0)
    out_b = np.concatenate([np.asarray(res.results[c]["out_b"])
                            for c in range(NCORE)], axis=0)
    return (out_a, out_b)


# revision 17
# speedup vs baseline: 1.2928x; 1.0017x over previous
"""Trainium2 Bass kernel for DepthwiseTensorProductModuleDict.

Computes, for each key k in {a, b}:
    w = MLP(edge_len_k)           # Linear(64->128) -> LayerNorm -> silu -> Linear(128->256)
    out_k = DTP(edge_fea_k, edge_vec_k, w)   # depthwise uvu tensor product

Sharding: edge dimension split across 8 NeuronCores (pure data parallel),
both dict keys processed by every core on its edge shard. Weights replicated.

Layout: edges packed 4 per partition -> macro tiles of 512 edges
[128 partitions, 4 slots, features]. Per-macro pipeline:
  PE (bf16): 2x len transpose -> mm1 (N=128 + fused-mean N=2) ->
             4x a transpose -> mm2 (N=256 + N=128)
  ACT: len->bf16 cast, PSUM->SBUF copies, Square+accum (sum h^2),
       Sqrt (std), Silu(scale,bias) for layernorm+silu fusion
  DVE: fast-reciprocal (rstd), LN stats, PSUM-coupled DTP elementwise
  GPSIMD: SBUF-only DTP elementwise (contiguous APs only)
"""
import os
import numpy as np

import concourse.bass as bass
import concourse.tile as tile
from concourse import bacc, mybir
from concourse.bass_utils import run_bass_kernel_spmd
from concourse.masks import make_identity

F32 = mybir.dt.float32
BF16 = mybir.dt.bfloat16
I32 = mybir.dt.int32
P = 128          # partitions
J = 4            # edges per partition
MACRO = P * J    # 512 edges per macro tile
E = 131072       # total edges per key
NCORE = 8
ESH = E // NCORE          # 16384 edges per core per key
NM = ESH // MACRO         # 32 macros per key per core
MUL = 64
FEA = 256
RAD = 64
HID = 128
EPS = 1e-5

_mult = mybir.AluOpType.mult
_add = mybir.AluOpType.add
_sub = mybir.AluOpType.subtract

# cached compiled program (host-side) keyed by per-key flags
_CACHE = {}

last_exec_time_ns = None
last_results = None


def _prep_weights(W1, b1, W2):
    """Host-side weight packing (bf16 for the PE path).

    Returns:
      w1m   [64, 128] bf16: W1
      m2    [64, 132] bf16: [mu_col | 0 | W1 W1^T] split hi|lo (PSUM-accumulated
            for ~16-bit mantissa: mean + sum-h^2 quadratic form)
      w2big [128, 384] bf16: [w1 | w2 | w3rep | w4]  (pre-scaled)
      b1_nz flag
    """
    inv_s2 = np.float32(1.0 / np.sqrt(np.float32(2.0)))
    inv_s3 = np.float32(1.0 / np.sqrt(np.float32(3.0)))
    import ml_dtypes
    bf = ml_dtypes.bfloat16

    b1_nz = bool(np.any(b1))
    W1bf = W1.astype(bf).astype(np.float32)          # round first for consistency
    w1m = W1bf.astype(bf)                            # [64, 128]
    mu_col = W1bf.mean(axis=1, keepdims=True)        # [64, 1]
    pad = np.zeros_like(mu_col)
    M = W1bf @ W1bf.T                                # [64, 64] quadratic form
    m2f = np.hstack([mu_col, pad, M]).astype(np.float32)   # [64, 66]
    m2hi = m2f.astype(bf)
    m2lo = (m2f - m2hi.astype(np.float32)).astype(bf)
    m2 = np.hstack([m2hi, m2lo])                           # [64, 132] bf16

    w1 = W2[:, 0:64] * inv_s2
    w2 = W2[:, 64:128] * inv_s2
    w3 = W2[:, 128:192] * inv_s2
    w4 = W2[:, 192:256] * (inv_s2 * inv_s3)
    w3rep = np.repeat(w3, 3, axis=1)                       # [128, 192]
    w2big = np.concatenate([w1, w2, w3rep, w4], axis=1).astype(bf)  # [128, 384]
    return w1m, m2, w2big, b1_nz


def _build_key(nc, tc, ctx, key, b1_nz, gbe_nz, ident_bf, magic8,
               eps_t, neps2_t, pools):
    """Emit instructions for one dict key's full shard (NM macros)."""
    fea = nc.dram_tensor(f"fea_{key}", [ESH, FEA], F32, kind="ExternalInput").ap()
    vec = nc.dram_tensor(f"vec_{key}", [ESH, 4], F32, kind="ExternalInput").ap()
    lng = nc.dram_tensor(f"len_{key}", [ESH, RAD], F32, kind="ExternalInput").ap()
    w1m_d = nc.dram_tensor(f"w1m_{key}", [RAD, HID], BF16,
                           kind="ExternalInput").ap()
    m2_d = nc.dram_tensor(f"m2_{key}", [RAD, 132], BF16,
                          kind="ExternalInput").ap()
    w2big_d = nc.dram_tensor(f"w2big_{key}", [HID, 384], BF16,
                             kind="ExternalInput").ap()
    out = nc.dram_tensor(f"out_{key}", [ESH, FEA], F32, kind="ExternalOutput").ap()
    b1_d = g_d = be_d = None
    if b1_nz:
        b1_d = nc.dram_tensor(f"b1_{key}", [HID], F32, kind="ExternalInput").ap()
    if gbe_nz:
        g_d = nc.dram_tensor(f"g_{key}", [HID], F32, kind="ExternalInput").ap()
        be_d = nc.dram_tensor(f"be_{key}", [HID], F32, kind="ExternalInput").ap()

    fea_v = fea.rearrange("(m p j) f -> m p j f", p=P, j=J)
    len_v = lng.rearrange("(m p j) f -> m p (j f)", p=P, j=J)   # [NM, 128, 256]
    out_v = out.rearrange("(m p j) f -> m p j f", p=P, j=J)
    vec_v = vec.rearrange("(m p j) f -> p m (j f)", p=P, j=J)   # [128, NM, 16]

    const = ctx.enter_context(tc.tile_pool(name=f"const_{key}", bufs=1))

    # --- weights ---
    w1m_sb = const.tile([RAD, HID], BF16)
    nc.sync.dma_start(out=w1m_sb, in_=w1m_d)
    m2_sb = const.tile([RAD, 132], BF16)
    nc.sync.dma_start(out=m2_sb, in_=m2_d)
    w2big_sb = const.tile([HID, 384], BF16)
    nc.sync.dma_start(out=w2big_sb, in_=w2big_d)

    b1rep = b1mu = grep = berep = None
    if b1_nz:
        b1mu_d = nc.dram_tensor(f"b1mu_{key}", [P, 1], F32,
                                kind="ExternalInput").ap()
        b1rep = const.tile([P, HID], F32)
        nc.sync.dma_start(out=b1rep, in_=b1_d.partition_broadcast(P))
        b1mu = const.tile([P, 1], F32)
        nc.sync.dma_start(out=b1mu, in_=b1mu_d)
    if gbe_nz:
        grep = const.tile([P, HID], F32)
        berep = const.tile([P, HID], F32)
        nc.sync.dma_start(out=grep, in_=g_d.partition_broadcast(P))
        nc.sync.dma_start(out=berep, in_=be_d.partition_broadcast(P))

    # --- whole-shard vec resident in SBUF ---
    vec_sb = const.tile([P, NM, J * 4], F32)
    nc.sync.dma_start(out=vec_sb, in_=vec_v)

    io, wk, st, ps_misc, ps_h, ps_wb = pools

    PAIR = 2
    for mp in range(NM // PAIR):
        ssq_p = st.tile([P, PAIR, J], F32, name="ssq_p")
        mus_p = st.tile([P, PAIR, J], F32, name="mus_p")
        saved = []
        for pm in range(PAIR):
            m = mp * PAIR + pm
            # ---------- loads ----------
            len_t = io.tile([P, J * RAD], F32, name="len_t")
            nc.sync.dma_start(out=len_t, in_=len_v[m])
            fea_t = io.tile([P, J, FEA], F32, name="fea_t")
            nc.sync.dma_start(out=fea_t, in_=fea_v[m])
            vrow = vec_sb[:, m, :].rearrange("p (j f) -> p j f", f=4)

            # ---------- len -> bf16, PE transpose, mm1 + [mu|M] ----------
            len_bf = wk.tile([P, J * RAD], BF16, name="len_bf")
            nc.scalar.copy(len_bf, len_t)
            lt_ps = ps_misc.tile([RAD, J * P], BF16, name="lt_ps", tag="misc")
            for j in range(J):
                nc.tensor.transpose(lt_ps[:, j * P:(j + 1) * P],
                                    len_bf[:, j * RAD:(j + 1) * RAD], ident_bf)
            lt_sb = wk.tile([RAD, J * P], BF16, name="lt_sb")
            nc.scalar.copy(lt_sb, lt_ps)

            h_ps = ps_h.tile([P, J, HID], F32, name="h_ps")
            mu_ps = ps_misc.tile([P, J, 66], F32, name="mu_ps", tag="misc")
            for j in range(J):
                slab = lt_sb[:, j * P:(j + 1) * P]
                nc.tensor.matmul(h_ps[:, j, :], slab, w1m_sb,
                                 start=True, stop=True)
                nc.tensor.matmul(mu_ps[:, j, :], slab, m2_sb[:, 0:66],
                                 start=True, stop=False)
                nc.tensor.matmul(mu_ps[:, j, :], slab, m2_sb[:, 66:132],
                                 start=False, stop=True)
            if b1_nz:
                hb = wk.tile([P, J, HID], F32, name="hb")
                nc.vector.tensor_tensor(
                    out=hb, in0=h_ps,
                    in1=b1rep.unsqueeze(1).broadcast_to([P, J, HID]), op=_add)
                h_src = hb
            else:
                h_src = h_ps

            # ssq = sum_i h_i^2 = sum_l (len @ M) * len   (quadratic form)
            qprod = wk.tile([P, J, RAD], F32, name="qprod")
            nc.vector.tensor_tensor(
                out=qprod, in0=mu_ps[:, :, 2:66],
                in1=len_bf.rearrange("p (j r) -> p j r", r=RAD), op=_mult)
            nc.vector.tensor_reduce(ssq_p[:, pm, :], qprod,
                                    axis=mybir.AxisListType.X,
                                    op=_add)
            nc.scalar.copy(mus_p[:, pm, :], mu_ps[:, :, 0:1].squeeze(2))
            saved.append((m, fea_t, h_src, vrow))

        # ---------- layernorm stats (batched across the pair) ----------
        if b1_nz:
            musb = st.tile([P, PAIR, J], F32, name="musb")
            nc.vector.tensor_tensor(out=musb, in0=mus_p,
                                    in1=b1mu.broadcast_to([P, PAIR, J]),
                                    op=_add)
            mus_p = musb
        musq = st.tile([P, PAIR, J], F32, name="musq")
        nc.scalar.activation(musq, mus_p, mybir.ActivationFunctionType.Square)
        var = st.tile([P, PAIR, J], F32, name="var")
        nc.vector.scalar_tensor_tensor(out=var, in0=ssq_p, scalar=1.0 / HID,
                                       in1=musq, op0=_mult, op1=_sub)
        # rstd via 1 Newton iteration from the fast-inverse-sqrt seed
        vpe = st.tile([P, PAIR, J], F32, name="vpe")
        nc.scalar.activation(vpe, var, mybir.ActivationFunctionType.Identity,
                             bias=eps_t[:, 0:1])
        nvpe = st.tile([P, PAIR, J], F32, name="nvpe")
        nc.scalar.activation(nvpe, var, mybir.ActivationFunctionType.Identity,
                             bias=neps2_t[:, 0:1], scale=-0.5)
        ibits = st.tile([P, PAIR, J], I32, name="ibits")
        nc.vector.tensor_scalar(out=ibits, in0=vpe.bitcast(I32), scalar1=1,
                                scalar2=None,
                                op0=mybir.AluOpType.logical_shift_right)
        seed = st.tile([P, PAIR, J], I32, name="seed")
        nc.vector.tensor_tensor(out=seed,
                                in0=magic8.rearrange("p (a j) -> p a j", a=2),
                                in1=ibits, op=_sub)
        y2 = st.tile([P, PAIR, J], F32, name="y2")
        nc.gpsimd.tensor_tensor(out=y2, in0=seed.bitcast(F32),
                                in1=seed.bitcast(F32), op=_mult)
        w_ = st.tile([P, PAIR, J], F32, name="w_")
        nc.gpsimd.tensor_tensor(out=w_, in0=y2, in1=nvpe, op=_mult)
        y_a = st.tile([P, PAIR, J], F32, name="y_a")
        nc.vector.scalar_tensor_tensor(out=y_a, in0=w_, scalar=1.5,
                                       in1=seed.bitcast(F32), op0=_add,
                                       op1=_mult)
        # second Newton iteration (cheap on GPS, keeps rstd ~1e-6)
        y2b = st.tile([P, PAIR, J], F32, name="y2b")
        nc.gpsimd.tensor_tensor(out=y2b, in0=y_a, in1=y_a, op=_mult)
        w2b_ = st.tile([P, PAIR, J], F32, name="w2b_")
        nc.gpsimd.tensor_tensor(out=w2b_, in0=y2b, in1=nvpe, op=_mult)
        rstd = st.tile([P, PAIR, J], F32, name="rstd")
        nc.vector.scalar_tensor_tensor(out=rstd, in0=w2b_, scalar=1.5,
                                       in1=y_a, op0=_add, op1=_mult)
        nbias = st.tile([P, PAIR, J], F32, name="nbias")
        nc.vector.scalar_tensor_tensor(out=nbias, in0=mus_p, scalar=-1.0,
                                       in1=rstd, op0=_mult, op1=_mult)

        for pm in range(PAIR):
            m, fea_t, h_src, vrow = saved[pm]
            # ---------- normalize + silu -> bf16 a ----------
            a_sb = wk.tile([P, J, HID], BF16, name="a_sb")
            if not gbe_nz:
                for j in range(J):
                    nc.scalar.activation(a_sb[:, j], h_src[:, j, :],
                                         mybir.ActivationFunctionType.Silu,
                                         bias=nbias[:, pm, j:j + 1],
                                         scale=rstd[:, pm, j:j + 1])
            else:
                hn = wk.tile([P, J, HID], F32, name="hn")
                for j in range(J):
                    nc.scalar.activation(hn[:, j], h_src[:, j, :],
                                         mybir.ActivationFunctionType.Identity,
                                         bias=nbias[:, pm, j:j + 1],
                                         scale=rstd[:, pm, j:j + 1])
                hg = wk.tile([P, J, HID], F32, name="hg")
                nc.vector.tensor_tensor(
                    out=hg, in0=hn,
                    in1=grep.unsqueeze(1).broadcast_to([P, J, HID]), op=_mult)
                nc.vector.tensor_tensor(
                    out=hg, in0=hg,
                    in1=berep.unsqueeze(1).broadcast_to([P, J, HID]), op=_add)
                for j in range(J):
                    nc.scalar.activation(a_sb[:, j], hg[:, j],
                                         mybir.ActivationFunctionType.Silu)

            # ---------- PE: transpose a, mm2 (single N=384) ----------
            at_ps = ps_misc.tile([P, J, HID], BF16, name="at_ps", tag="misc")
            for j in range(J):
                nc.tensor.transpose(at_ps[:, j, :], a_sb[:, j, :], ident_bf)
            at_sb = wk.tile([P, J, HID], BF16, name="at_sb")
            nc.scalar.copy(at_sb, at_ps)

            wb = ps_wb.tile([P, J, 384], F32, name="wb",
                            padded_shape=[P, J, 512])
            for j in range(J):
                nc.tensor.matmul(wb[:, j, :], at_sb[:, j, :], w2big_sb,
                                 start=True, stop=True)

            # ---------- DTP ----------
            out_t = io.tile([P, J, FEA], F32, name="out_t")
            x0 = fea_t[:, :, 0:MUL]                    # [P,J,64]
            x1 = fea_t[:, :, MUL:FEA]                  # [P,J,192]
            x1v = x1.rearrange("p j (u d) -> p j u d", d=3)
            y1b = vrow[:, :, 1:4].unsqueeze(2).broadcast_to([P, J, MUL, 3])

            # b = x1 * y1b  (GPS)
            b_t = wk.tile([P, J, MUL, 3], F32, name="b_t")
            nc.gpsimd.tensor_tensor(out=b_t, in0=x1v, in1=y1b, op=_mult)
            # d = sum_d b   (strided adds: DVE + GPS)
            d_t = wk.tile([P, J, MUL], F32, name="d_t")
            nc.vector.tensor_tensor(out=d_t, in0=b_t[:, :, :, 0],
                                    in1=b_t[:, :, :, 1], op=_add)
            d2_t = wk.tile([P, J, MUL], F32, name="d2_t")
            nc.gpsimd.tensor_tensor(out=d2_t, in0=d_t, in1=b_t[:, :, :, 2],
                                    op=_add)

            # t2 = w2' * x0   (DVE, PSUM in0)
            t2 = wk.tile([P, J, MUL], F32, name="t2")
            nc.vector.tensor_tensor(out=t2, in0=wb[:, :, 64:128], in1=x0,
                                    op=_mult)
            # m1y = (x0*y0)*w1'  (GPS broadcast mult + batched DVE TT)
            p1 = wk.tile([P, J, MUL], F32, name="p1")
            nc.gpsimd.tensor_tensor(
                out=p1, in0=x0,
                in1=vrow[:, :, 0:1].broadcast_to([P, J, MUL]), op=_mult)
            m1y = wk.tile([P, J, MUL], F32, name="m1y")
            nc.vector.tensor_tensor(out=m1y, in0=p1, in1=wb[:, :, 0:64],
                                    op=_mult)
            # md = d * w4'  (DVE, PSUM in1)
            md = wk.tile([P, J, MUL], F32, name="md")
            nc.vector.tensor_tensor(out=md, in0=d2_t, in1=wb[:, :, 320:384],
                                    op=_mult)
            # g = (x1 * y0) * w3rep'  (DVE STT per j; wb is PSUM)
            g_t = wk.tile([P, J, MUL * 3], F32, name="g_t")
            for j in range(J):
                nc.vector.scalar_tensor_tensor(
                    out=g_t[:, j], in0=x1[:, j], scalar=vrow[:, j, 0:1],
                    in1=wb[:, j, 128:320], op0=_mult, op1=_mult)

            # e = t2b * y1b  (GPS, double-broadcast)
            e_t = wk.tile([P, J, MUL, 3], F32, name="e_t")
            nc.gpsimd.tensor_tensor(
                out=e_t, in0=t2.unsqueeze(3).broadcast_to([P, J, MUL, 3]),
                in1=y1b, op=_mult)

            # out1 = e + g  (DVE); out0 = m1y + md  (GPS)
            nc.vector.tensor_tensor(
                out=out_t[:, :, MUL:FEA],
                in0=e_t.rearrange("p j u d -> p j (u d)"), in1=g_t, op=_add)
            nc.gpsimd.tensor_tensor(out=out_t[:, :, 0:MUL], in0=m1y, in1=md,
                                    op=_add)

            # ---------- store ----------
            nc.sync.dma_start(out=out_v[m], in_=out_t)


def _build_program(flags):
    """flags = {key: (b1_nz, gbe_nz)}"""
    import contextlib
    nc = bacc.Bacc("TRN2", target_bir_lowering=False, debug=False)
    with tile.TileContext(nc) as tc:
        with contextlib.ExitStack() as ctx:
            glob = ctx.enter_context(tc.tile_pool(name="glob", bufs=1))
            ident = glob.tile([P, P], F32)
            make_identity(nc, ident)
            ident_bf = glob.tile([P, P], BF16)
            nc.scalar.copy(ident_bf, ident)
            magic8 = glob.tile([P, 2 * J], I32)
            nc.vector.memset(magic8, 0x5F3759DF)
            eps_t = glob.tile([P, 1], F32)
            nc.vector.memset(eps_t, EPS)
            neps2_t = glob.tile([P, 1], F32)
            nc.vector.memset(neps2_t, -EPS / 2)
            pools = (
                ctx.enter_context(tc.tile_pool(name="io", bufs=4)),
                ctx.enter_context(tc.tile_pool(name="wk", bufs=3)),
                ctx.enter_context(tc.tile_pool(name="st", bufs=4)),
                ctx.enter_context(tc.tile_pool(name="psmisc", bufs=2,
                                               space="PSUM")),
                ctx.enter_context(tc.tile_pool(name="psh", bufs=2, space="PSUM")),
                ctx.enter_context(tc.tile_pool(name="pswb", bufs=1, space="PSUM")),
            )
            for key in ("a", "b"):
                b1_nz, gbe_nz = flags[key]
                _build_key(nc, tc, ctx, key, b1_nz, gbe_nz, ident_bf, magic8,
                           eps_t, neps2_t, pools)
    nc.compile()
    return nc


def kernel(edge_fea_a, edge_vec_a, edge_len_a, W1_a, b1_a, g_a, be_a, W2_a,
           edge_fea_b, edge_vec_b, edge_len_b, W1_b, b1_b, g_b, be_b, W2_b):
    global last_exec_time_ns, last_results
    ins = {
        "a": (edge_fea_a, edge_vec_a, edge_len_a, W1_a, b1_a, g_a, be_a, W2_a),
        "b": (edge_fea_b, edge_vec_b, edge_len_b, W1_b, b1_b, g_b, be_b, W2_b),
    }
    prepped = {}
    flags = {}
    for key, (fea, vec, lng, W1, b1, g, be, W2) in ins.items():
        w1m, m2, w2big, b1_nz = _prep_weights(
            np.asarray(W1, np.float32), np.asarray(b1, np.float32),
            np.asarray(W2, np.float32))
        gbe_nz = bool(np.any(np.asarray(g) != 1.0) or np.any(np.asarray(be)))
        prepped[key] = (w1m, m2, w2big)
        flags[key] = (b1_nz, gbe_nz)

    ck = tuple(flags[k] for k in ("a", "b"))
    if ck not in _CACHE:
        _CACHE[ck] = _build_program(flags)
    nc = _CACHE[ck]

    in_maps = []
    for c in range(NCORE):
        sl = slice(c * ESH, (c + 1) * ESH)
        m = {}
        for key, (fea, vec, lng, W1, b1, g, be, W2) in ins.items():
            m[f"fea_{key}"] = np.ascontiguousarray(np.asarray(fea, np.float32)[sl])
            m[f"vec_{key}"] = np.ascontiguousarray(np.asarray(vec, np.float32)[sl])
            m[f"len_{key}"] = np.ascontiguousarray(np.asarray(lng, np.float32)[sl])
            m[f"w1m_{key}"] = prepped[key][0]
            m[f"m2_{key}"] = prepped[key][1]
            m[f"w2big_{key}"] = prepped[key][2]
            if flags[key][0]:
                m[f"b1_{key}"] = np.asarray(b1, np.float32)
                m[f"b1mu_{key}"] = np.full(
                    (P, 1), np.asarray(b1, np.float32).mean(), np.float32)
            if flags[key][1]:
                m[f"g_{key}"] = np.asarray(g, np.float32)
                m[f"be_{key}"] = np.asarray(be, np.float32)
        in_maps.append(m)

    trace = bool(int(os.environ.get("KERNEL_TRACE", "0")))
    res = run_bass_kernel_spmd(nc, in_maps, list(range(NCORE)), trace=trace)
    globals()["last_results"] = res
    last_exec_time_ns = res.exec_time_ns

    out_a = np.concatenate([np.asarray(res.results[c]["out_a"])
                            for c in range(NCORE)], axis=0)
    out_b = np.concatenate([np.asarray(res.results[c]["out_b"])
                            for c in range(NCORE)], axis=0)
    return (out_a, out_b)
